# revision 12
# baseline (speedup 1.0000x reference)
"""Multi-head self-attention (B=4, L=2048, D=1024, H=16, Hd=64) on 8 TRN2 NeuronCores.

Sharding: data-parallel over batch (4) x tensor-parallel over head-groups (2).
Core c handles batch c//2 with heads [8*(c%2), 8*(c%2)+8). Each core computes a
partial out-projection over its 512 head-dims; the host sums the two partials
per batch and adds the fused bias (bo + bv @ wo.T, exact because softmax rows
sum to 1, so the v-bias passes through attention unchanged).

Per-core kernel (bf16 compute, f32 accumulation):
  - q/k feature-major [512, 2048] = w.T-slice @ x.T (bias per-partition, q
    pre-scaled by 1/sqrt(Hd) via host-scaled weights)
  - v token-major [2048, 8*65] with a ones-column per head: the attn@v matmul
    (lhsT = v_aug) then yields softmax denominators in PSUM row 64 for free
  - scores S^T[key, q] per head via K=64 matmuls, two heads packed into the
    128x128 PE array via base-partition 0/64 row tiling
  - exp on ScalarE (no max-subtraction: scores are ~N(0,1), fp32-safe)
  - normalize: DVE reciprocal of row 64 -> PE ones-broadcast -> DVE scale
  - out-projection token-major, host adds partials
"""
import os
import sys
import time
from contextlib import ExitStack

sys.path.insert(0, "/opt/trn_rl_repo")

import numpy as np
import ml_dtypes

import concourse.bass as bass
import concourse.tile as tile
from concourse import bacc, mybir
from concourse.bass_utils import run_bass_kernel_spmd

BF16 = mybir.dt.bfloat16
F32 = mybir.dt.float32
NPBF16 = ml_dtypes.bfloat16

B, L, D = 4, 2048, 1024
H, HD = 16, 64
HPC = 8            # heads per core
DHC = HPC * HD     # 512 local head-dims per core
NCORES = 8

NKD = D // 128     # 8 contraction tiles over model dim
NTT = L // 128     # 16 token tiles of 128
NQT = L // 512     # 4 query tiles of 512
NKT = L // 128     # 16 key tiles of 128
NHP = HPC // 2     # 4 head pairs

_NC_CACHE = None
LAST_RESULTS = None  # test harness introspection


def _emit(nc, tc, ctx):
    EXP = mybir.ActivationFunctionType.Exp
    ADD = mybir.AluOpType.add
    from collections import deque

    xT = nc.declare_dram_parameter("xT", [D, L], BF16, isOutput=False)
    wqT = nc.declare_dram_parameter("wqT", [D, DHC], BF16, isOutput=False)
    wkT = nc.declare_dram_parameter("wkT", [D, DHC], BF16, isOutput=False)
    wvT = nc.declare_dram_parameter("wvT", [D, DHC], BF16, isOutput=False)
    woT = nc.declare_dram_parameter("woT", [DHC, D], BF16, isOutput=False)
    bq = nc.declare_dram_parameter("bq", [DHC, 1], F32, isOutput=False)
    bk = nc.declare_dram_parameter("bk", [DHC, 1], F32, isOutput=False)
    out = nc.declare_dram_parameter("out", [L, D], F32, isOutput=True)

    p_xt = ctx.enter_context(tc.tile_pool(name="xt", bufs=NKD))
    p_wq = ctx.enter_context(tc.tile_pool(name="wq", bufs=NKD))
    p_wk = ctx.enter_context(tc.tile_pool(name="wk", bufs=NKD))
    p_wv = ctx.enter_context(tc.tile_pool(name="wv", bufs=NKD))
    p_wo = ctx.enter_context(tc.tile_pool(name="wo", bufs=4))
    p_bias = ctx.enter_context(tc.tile_pool(name="bias", bufs=2 * NHP + 1))
    p_q = ctx.enter_context(tc.tile_pool(name="q", bufs=NHP))
    p_k = ctx.enter_context(tc.tile_pool(name="k", bufs=NHP))
    p_v = ctx.enter_context(tc.tile_pool(name="v", bufs=NTT))
    p_e = ctx.enter_context(tc.tile_pool(name="e", bufs=8))
    p_ost = ctx.enter_context(tc.tile_pool(name="ost", bufs=NQT + 1))
    p_rcp = ctx.enter_context(tc.tile_pool(name="rcp", bufs=2))
    p_rb = ctx.enter_context(tc.tile_pool(name="rb", bufs=2))
    p_outst = ctx.enter_context(tc.tile_pool(name="outst", bufs=3))
    # PSUM: scores 2x[128,1024] (4 banks) + attnv accum 2x[128,512] (2 banks)
    # + one background ring [128,1024] (2 banks) = 8 banks exactly.
    p_mm = ctx.enter_context(tc.tile_pool(name="pmm", bufs=2, space="PSUM"))
    p_acc = ctx.enter_context(tc.tile_pool(name="pacc", bufs=2, space="PSUM"))
    p_bg = ctx.enter_context(tc.tile_pool(name="pbg", bufs=1, space="PSUM"))

    # --- weight / input DMAs ---
    xt = [p_xt.tile([128, L], BF16, tag="xt", name=f"xt{i}") for i in range(NKD)]
    wqs = [p_wq.tile([128, DHC], BF16, tag="wq", name=f"wqs{i}") for i in range(NKD)]
    wks = [p_wk.tile([128, DHC], BF16, tag="wk", name=f"wks{i}") for i in range(NKD)]
    wvs = [p_wv.tile([128, DHC], BF16, tag="wv", name=f"wvs{i}") for i in range(NKD)]
    wos = [p_wo.tile([128, D], BF16, tag="wo", name=f"wos{i}") for i in range(4)]
    for kd in range(NKD):
        nc.gpsimd.dma_start(xt[kd][:], xT[kd * 128:(kd + 1) * 128, :])
        nc.gpsimd.dma_start(wqs[kd][:], wqT[kd * 128:(kd + 1) * 128, :])
        nc.gpsimd.dma_start(wks[kd][:], wkT[kd * 128:(kd + 1) * 128, :])
    for kd in range(NKD):
        nc.gpsimd.dma_start(wvs[kd][:], wvT[kd * 128:(kd + 1) * 128, :])
    for j in range(4):
        nc.gpsimd.dma_start(wos[j][:], woT[j * 128:(j + 1) * 128, :])
    bqt, bkt = [], []
    for hp in range(NHP):
        tq = p_bias.tile([128, 1], F32, tag="bias")
        tk = p_bias.tile([128, 1], F32, tag="bias")
        nc.gpsimd.dma_start(tq[:], bq[hp * 128:(hp + 1) * 128, :])
        nc.gpsimd.dma_start(tk[:], bk[hp * 128:(hp + 1) * 128, :])
        bqt.append(tq)
        bkt.append(tk)
    ones_sb = p_bias.tile([128, 64], F32, tag="bias")
    nc.vector.memset(ones_sb[:], 1.0)

    q_t, k_t = [None] * NHP, [None] * NHP
    v_t = [None] * NTT
    outst_q = {}

    def emit_qk_unit(hp, tt, pool):
        if q_t[hp] is None:
            q_t[hp] = p_q.tile([128, L], BF16, tag="q", name=f"qT{hp}")
            k_t[hp] = p_k.tile([128, L], BF16, tag="k", name=f"kT{hp}")
        qt_, kt_ = q_t[hp], k_t[hp]
        ps = pool.tile([128, 1024], F32, tag=pool.name[1:], name=f"psqk{hp}_{tt}")
        for kd in range(NKD):
            nc.tensor.matmul(
                ps[:, 0:512], wqs[kd][:, hp * 128:(hp + 1) * 128],
                xt[kd][:, tt * 512:(tt + 1) * 512],
                start=(kd == 0), stop=(kd == NKD - 1),
            )
        for kd in range(NKD):
            nc.tensor.matmul(
                ps[:, 512:1024], wks[kd][:, hp * 128:(hp + 1) * 128],
                xt[kd][:, tt * 512:(tt + 1) * 512],
                start=(kd == 0), stop=(kd == NKD - 1),
            )
        nc.vector.tensor_scalar(
            qt_[:, tt * 512:(tt + 1) * 512], ps[:, 0:512], bqt[hp][:], None, ADD)
        nc.vector.tensor_scalar(
            kt_[:, tt * 512:(tt + 1) * 512], ps[:, 512:1024], bkt[hp][:], None, ADD)

    def emit_v(t, pool):
        ps = pool.tile([128, 1024], F32, tag=pool.name[1:], name=f"psv{t}")
        for kd in range(NKD):
            nc.tensor.matmul(
                ps[:, 0:512], xt[kd][:, t * 128:(t + 1) * 128], wvs[kd][:],
                start=(kd == 0), stop=(kd == NKD - 1),
            )
        vt = p_v.tile([128, HPC, HD + 1], BF16, tag="v", name=f"v{t}")
        nc.vector.memset(vt[:, :, HD:HD + 1], 1.0)
        nc.vector.tensor_copy(
            vt[:, :, 0:HD], ps[:, 0:512].rearrange("p (h d) -> p h d", h=HPC))
        v_t[t] = vt

    def emit_outproj_chunk(qt, ost, tl, half):
        key = (qt, tl)
        if key not in outst_q:
            outst_q[key] = p_outst.tile(
                [128, 1024], F32, tag="outst", name=f"outst{qt}_{tl}")
        outst = outst_q[key]
        ps_o = p_bg.tile([128, 1024], F32, tag="bg", name=f"pso{qt}_{tl}_{half}")
        for j in range(NHP):
            nc.tensor.matmul(
                ps_o[:, 0:512],
                ost[j][:, tl * 128:(tl + 1) * 128],
                wos[j][:, half * 512:(half + 1) * 512],
                start=(j == 0), stop=(j == NHP - 1),
            )
        nc.vector.tensor_copy(outst[:, half * 512:(half + 1) * 512], ps_o[:, 0:512])
        if half == 1:
            t = qt * 4 + tl
            nc.gpsimd.dma_start(out[t * 128:(t + 1) * 128, :], outst[:])

    bg = deque()

    def _pre_kt(kt):
        if bg:
            bg.popleft()()

    def emit_attn(qt, hp):
        po_a = p_acc.tile([128, 512], F32, tag="acc", name=f"poa{qt}_{hp}")
        po_b = p_acc.tile([128, 512], F32, tag="acc", name=f"pob{qt}_{hp}")
        for kt in range(NKT):
            _pre_kt(kt)
            # scores + exp are the ACT-feeding critical path: lift their
            # priority above backlogged attnv / background work so the
            # scheduler never starves ScalarE behind lower-value matmuls.
            with tc.high_priority(offset=1200):
                ps = p_mm.tile([128, 1024], F32, tag="mm", name=f"pss{qt}_{hp}_{kt}")
                nc.tensor.matmul(
                    ps[:, 0:512],
                    k_t[hp][0:64, kt * 128:(kt + 1) * 128],
                    q_t[hp][0:64, qt * 512:(qt + 1) * 512],
                    start=True, stop=True,
                )
                nc.tensor.matmul(
                    ps[:, 512:1024],
                    k_t[hp][64:128, kt * 128:(kt + 1) * 128],
                    q_t[hp][64:128, qt * 512:(qt + 1) * 512],
                    start=True, stop=True,
                )
                e = p_e.tile([128, 1024], BF16, tag="e", name=f"e{qt}_{hp}_{kt}")
                nc.scalar.activation(e[:], ps[:], EXP)
            nc.tensor.matmul(
                po_a[0:65, :], v_t[kt][:, 2 * hp, :], e[:, 0:512],
                start=(kt == 0), stop=(kt == NKT - 1),
            )
            nc.tensor.matmul(
                po_b[0:65, :], v_t[kt][:, 2 * hp + 1, :], e[:, 512:1024],
                start=(kt == 0), stop=(kt == NKT - 1),
            )
        o = p_ost.tile([128, 512], BF16, tag=f"ost{hp}", name=f"ost{qt}_{hp}")
        for half, po in ((0, po_a), (1, po_b)):
            # approx_fast mishandles nonzero base partitions -> run over
            # [0:65] (base 0) and consume row 64 only; rows 0:64 are junk.
            rcp = p_rcp.tile([128, 512], F32, tag="rcp", name=f"rcp{qt}_{hp}_{half}")
            nc.vector.reciprocal_approx_fast(out=rcp[0:65, :], in_=po[0:65, :])
            # broadcast recip into the po bank's unused partitions 64:128
            # (row 64's sums are already consumed by the reciprocal)
            nc.tensor.matmul(
                po[64:128, :], ones_sb[64:65, :], rcp[64:65, :],
                start=True, stop=True, tile_position=(64, 64),
                skip_group_check=True,
            )
            rb = p_rb.tile([64, 512], F32, tag="rb", name=f"rb{qt}_{hp}_{half}")
            nc.vector.tensor_copy(rb[:], po[64:128, :])
            nc.vector.tensor_mul(o[64 * half:64 * half + 64, :], po[0:64, :], rb[:])
        return o

    # --- emission: jit prefix, hp-outer rows, spaced background trickle ---
    ost_q = [[None] * NHP for _ in range(NQT)]

    def _pre_kt(kt):
        while bg:
            u = bg.popleft()
            if u is not None:
                u()
            break

    def spaced(units, gap):
        seq = []
        for u in units:
            seq.append(u)
            seq.extend([None] * gap)
        return seq

    emit_qk_unit(0, 0, p_mm)
    for t in range(3):
        emit_v(t, p_mm)
    qk0 = [(lambda tt=tt: emit_qk_unit(0, tt, p_bg)) for tt in range(1, NQT)]
    vs = {t: (lambda t=t: emit_v(t, p_bg)) for t in range(3, NTT)}
    # deadline order: v(t) must be EMITTED by slot t; qk0 tt_j by slot 4j
    bg.extend([vs[3], vs[4], qk0[0], vs[5], vs[6], vs[7], qk0[1], vs[8],
               vs[9], vs[10], vs[11], qk0[2], vs[12], vs[13], vs[14], vs[15]])
    bg.extend(spaced([(lambda tt=tt: emit_qk_unit(1, tt, p_bg))
                      for tt in range(NQT)], 3))
    for qt in range(NQT):
        ost_q[qt][0] = emit_attn(qt, 0)
    bg.extend(spaced([(lambda tt=tt: emit_qk_unit(2, tt, p_bg))
                      for tt in range(NQT)], 6))
    for qt in range(NQT):
        ost_q[qt][1] = emit_attn(qt, 1)
    bg.extend(spaced([(lambda tt=tt: emit_qk_unit(3, tt, p_bg))
                      for tt in range(NQT)], 6))
    for qt in range(NQT):
        ost_q[qt][2] = emit_attn(qt, 2)
    for qt in range(NQT):
        ost_q[qt][3] = emit_attn(qt, 3)
        bg.extend(spaced([
            (lambda q=qt, tl=tl, half=half:
             emit_outproj_chunk(q, ost_q[q], tl, half))
            for tl in range(4) for half in range(2)], 1))
    while bg:
        u = bg.popleft()
        if u is not None:
            u()


def _build_nc():
    nc = bacc.Bacc("TRN2", target_bir_lowering=False, debug=False, num_devices=NCORES)
    with tile.TileContext(nc) as tc, ExitStack() as ctx:
        _emit(nc, tc, ctx)
    nc.compile()
    return nc


def kernel(inputs, wq, bq, wk, bk, wv, bv, wo, bo):
    global _NC_CACHE, LAST_RESULTS
    if _NC_CACHE is None:
        _NC_CACHE = _build_nc()
    nc = _NC_CACHE

    inputs = np.asarray(inputs, dtype=np.float32)
    wq, bq, wk, bk = (np.asarray(a, dtype=np.float32) for a in (wq, bq, wk, bk))
    wv, bv, wo, bo = (np.asarray(a, dtype=np.float32) for a in (wv, bv, wo, bo))

    scale = np.float32(1.0 / np.sqrt(HD))
    wqT_full = np.ascontiguousarray(wq.T) * scale
    wkT_full = np.ascontiguousarray(wk.T)
    wvT_full = np.ascontiguousarray(wv.T)
    woT_full = np.ascontiguousarray(wo.T)

    in_maps = []
    for c in range(NCORES):
        b, g = c // 2, c % 2
        cols = slice(g * DHC, (g + 1) * DHC)
        in_maps.append({
            "xT": np.ascontiguousarray(inputs[b].T).astype(NPBF16),
            "wqT": np.ascontiguousarray(wqT_full[:, cols]).astype(NPBF16),
            "wkT": np.ascontiguousarray(wkT_full[:, cols]).astype(NPBF16),
            "wvT": np.ascontiguousarray(wvT_full[:, cols]).astype(NPBF16),
            "woT": np.ascontiguousarray(woT_full[cols, :]).astype(NPBF16),
            "bq": (bq[cols] * scale).astype(np.float32).reshape(DHC, 1),
            "bk": bk[cols].astype(np.float32).reshape(DHC, 1),
        })

    trace = bool(int(os.environ.get("BASS_KERNEL_TRACE", "0")))
    res = run_bass_kernel_spmd(nc, in_maps, core_ids=list(range(NCORES)), trace=trace)
    LAST_RESULTS = res

    fused_bias = (bo + bv @ wo.T).astype(np.float32)
    outv = np.empty((B, L, D), dtype=np.float32)
    for b in range(B):
        outv[b] = res.results[2 * b]["out"] + res.results[2 * b + 1]["out"] + fused_bias
    return outv


# revision 13
# speedup vs baseline: 1.0067x; 1.0067x over previous
"""Multi-head self-attention (B=4, L=2048, D=1024, H=16, Hd=64) on 8 TRN2 NeuronCores.

Sharding: data-parallel over batch (4) x tensor-parallel over head-groups (2).
Core c handles batch c//2 with heads [8*(c%2), 8*(c%2)+8). Each core computes a
partial out-projection over its 512 head-dims; the host sums the two partials
per batch and adds the fused bias (bo + bv @ wo.T, exact because softmax rows
sum to 1, so the v-bias passes through attention unchanged).

Per-core kernel (bf16 compute, f32 accumulation):
  - q/k feature-major [512, 2048] = w.T-slice @ x.T (bias per-partition, q
    pre-scaled by 1/sqrt(Hd) via host-scaled weights)
  - v token-major [2048, 8*65] with a ones-column per head: the attn@v matmul
    (lhsT = v_aug) then yields softmax denominators in PSUM row 64 for free
  - scores S^T[key, q] per head via K=64 matmuls, two heads packed into the
    128x128 PE array via base-partition 0/64 row tiling
  - exp on ScalarE (no max-subtraction: scores are ~N(0,1), fp32-safe)
  - normalize: DVE reciprocal of row 64 -> PE ones-broadcast -> DVE scale
  - out-projection token-major, host adds partials
"""
import os
import sys
import time
from contextlib import ExitStack

sys.path.insert(0, "/opt/trn_rl_repo")

import numpy as np
import ml_dtypes

import concourse.bass as bass
import concourse.tile as tile
from concourse import bacc, mybir
from concourse.bass_utils import run_bass_kernel_spmd

BF16 = mybir.dt.bfloat16
F32 = mybir.dt.float32
NPBF16 = ml_dtypes.bfloat16

B, L, D = 4, 2048, 1024
H, HD = 16, 64
HPC = 8            # heads per core
DHC = HPC * HD     # 512 local head-dims per core
NCORES = 8

NKD = D // 128     # 8 contraction tiles over model dim
NTT = L // 128     # 16 token tiles of 128
NQT = L // 512     # 4 query tiles of 512
NKT = L // 128     # 16 key tiles of 128
NHP = HPC // 2     # 4 head pairs

_NC_CACHE = None
LAST_RESULTS = None  # test harness introspection


def _emit(nc, tc, ctx):
    EXP = mybir.ActivationFunctionType.Exp
    ADD = mybir.AluOpType.add
    from collections import deque

    xT = nc.declare_dram_parameter("xT", [D, L], BF16, isOutput=False)
    wqT = nc.declare_dram_parameter("wqT", [D, DHC], BF16, isOutput=False)
    wkT = nc.declare_dram_parameter("wkT", [D, DHC], BF16, isOutput=False)
    wvT = nc.declare_dram_parameter("wvT", [D, DHC], BF16, isOutput=False)
    woT = nc.declare_dram_parameter("woT", [DHC, D], BF16, isOutput=False)
    bq = nc.declare_dram_parameter("bq", [DHC, 1], F32, isOutput=False)
    bk = nc.declare_dram_parameter("bk", [DHC, 1], F32, isOutput=False)
    out = nc.declare_dram_parameter("out", [L, D], F32, isOutput=True)

    p_xt = ctx.enter_context(tc.tile_pool(name="xt", bufs=NKD))
    p_wq = ctx.enter_context(tc.tile_pool(name="wq", bufs=NKD))
    p_wk = ctx.enter_context(tc.tile_pool(name="wk", bufs=NKD))
    p_wv = ctx.enter_context(tc.tile_pool(name="wv", bufs=NKD))
    p_wo = ctx.enter_context(tc.tile_pool(name="wo", bufs=4))
    p_bias = ctx.enter_context(tc.tile_pool(name="bias", bufs=2 * NHP + 1))
    p_q = ctx.enter_context(tc.tile_pool(name="q", bufs=NHP))
    p_k = ctx.enter_context(tc.tile_pool(name="k", bufs=NHP))
    p_v = ctx.enter_context(tc.tile_pool(name="v", bufs=NTT))
    p_e = ctx.enter_context(tc.tile_pool(name="e", bufs=8))
    p_ost = ctx.enter_context(tc.tile_pool(name="ost", bufs=NQT + 1))
    p_rcp = ctx.enter_context(tc.tile_pool(name="rcp", bufs=2))
    p_rb = ctx.enter_context(tc.tile_pool(name="rb", bufs=2))
    p_outst = ctx.enter_context(tc.tile_pool(name="outst", bufs=3))
    # PSUM: scores 2x[128,1024] (4 banks) + attnv accum 2x[128,512] (2 banks)
    # + one background ring [128,1024] (2 banks) = 8 banks exactly.
    p_mm = ctx.enter_context(tc.tile_pool(name="pmm", bufs=2, space="PSUM"))
    p_acc = ctx.enter_context(tc.tile_pool(name="pacc", bufs=2, space="PSUM"))
    p_bg = ctx.enter_context(tc.tile_pool(name="pbg", bufs=1, space="PSUM"))

    # --- weight / input DMAs ---
    xt = [p_xt.tile([128, L], BF16, tag="xt", name=f"xt{i}") for i in range(NKD)]
    wqs = [p_wq.tile([128, DHC], BF16, tag="wq", name=f"wqs{i}") for i in range(NKD)]
    wks = [p_wk.tile([128, DHC], BF16, tag="wk", name=f"wks{i}") for i in range(NKD)]
    wvs = [p_wv.tile([128, DHC], BF16, tag="wv", name=f"wvs{i}") for i in range(NKD)]
    wos = [p_wo.tile([128, D], BF16, tag="wo", name=f"wos{i}") for i in range(4)]
    for kd in range(NKD):
        nc.gpsimd.dma_start(xt[kd][:], xT[kd * 128:(kd + 1) * 128, :])
        nc.gpsimd.dma_start(wqs[kd][:], wqT[kd * 128:(kd + 1) * 128, :])
        nc.gpsimd.dma_start(wks[kd][:], wkT[kd * 128:(kd + 1) * 128, :])
    for kd in range(NKD):
        nc.gpsimd.dma_start(wvs[kd][:], wvT[kd * 128:(kd + 1) * 128, :])
    for j in range(4):
        nc.gpsimd.dma_start(wos[j][:], woT[j * 128:(j + 1) * 128, :])
    bqt, bkt = [], []
    for hp in range(NHP):
        tq = p_bias.tile([128, 1], F32, tag="bias")
        tk = p_bias.tile([128, 1], F32, tag="bias")
        nc.gpsimd.dma_start(tq[:], bq[hp * 128:(hp + 1) * 128, :])
        nc.gpsimd.dma_start(tk[:], bk[hp * 128:(hp + 1) * 128, :])
        bqt.append(tq)
        bkt.append(tk)
    ones_sb = p_bias.tile([128, 64], F32, tag="bias")
    nc.vector.memset(ones_sb[:], 1.0)

    q_t, k_t = [None] * NHP, [None] * NHP
    v_t = [None] * NTT
    outst_q = {}

    def emit_qk_unit(hp, tt, pool):
        if q_t[hp] is None:
            q_t[hp] = p_q.tile([128, L], BF16, tag="q", name=f"qT{hp}")
            k_t[hp] = p_k.tile([128, L], BF16, tag="k", name=f"kT{hp}")
        qt_, kt_ = q_t[hp], k_t[hp]
        ps = pool.tile([128, 1024], F32, tag=pool.name[1:], name=f"psqk{hp}_{tt}")
        for kd in range(NKD):
            nc.tensor.matmul(
                ps[:, 0:512], wqs[kd][:, hp * 128:(hp + 1) * 128],
                xt[kd][:, tt * 512:(tt + 1) * 512],
                start=(kd == 0), stop=(kd == NKD - 1),
            )
        for kd in range(NKD):
            nc.tensor.matmul(
                ps[:, 512:1024], wks[kd][:, hp * 128:(hp + 1) * 128],
                xt[kd][:, tt * 512:(tt + 1) * 512],
                start=(kd == 0), stop=(kd == NKD - 1),
            )
        nc.vector.tensor_scalar(
            qt_[:, tt * 512:(tt + 1) * 512], ps[:, 0:512], bqt[hp][:], None, ADD)
        nc.vector.tensor_scalar(
            kt_[:, tt * 512:(tt + 1) * 512], ps[:, 512:1024], bkt[hp][:], None, ADD)

    def emit_v(t, pool):
        ps = pool.tile([128, 1024], F32, tag=pool.name[1:], name=f"psv{t}")
        for kd in range(NKD):
            nc.tensor.matmul(
                ps[:, 0:512], xt[kd][:, t * 128:(t + 1) * 128], wvs[kd][:],
                start=(kd == 0), stop=(kd == NKD - 1),
            )
        vt = p_v.tile([128, HPC, HD + 1], BF16, tag="v", name=f"v{t}")
        nc.vector.memset(vt[:, :, HD:HD + 1], 1.0)
        nc.vector.tensor_copy(
            vt[:, :, 0:HD], ps[:, 0:512].rearrange("p (h d) -> p h d", h=HPC))
        v_t[t] = vt

    def emit_outproj_chunk(qt, ost, tl, half):
        key = (qt, tl)
        if key not in outst_q:
            outst_q[key] = p_outst.tile(
                [128, 1024], F32, tag="outst", name=f"outst{qt}_{tl}")
        outst = outst_q[key]
        ps_o = p_bg.tile([128, 1024], F32, tag="bg", name=f"pso{qt}_{tl}_{half}")
        for j in range(NHP):
            nc.tensor.matmul(
                ps_o[:, 0:512],
                ost[j][:, tl * 128:(tl + 1) * 128],
                wos[j][:, half * 512:(half + 1) * 512],
                start=(j == 0), stop=(j == NHP - 1),
            )
        nc.vector.tensor_copy(outst[:, half * 512:(half + 1) * 512], ps_o[:, 0:512])
        if half == 1:
            t = qt * 4 + tl
            nc.gpsimd.dma_start(out[t * 128:(t + 1) * 128, :], outst[:])

    bg = deque()

    def _pre_kt(kt):
        if bg:
            bg.popleft()()

    def emit_attn(qt, hp):
        po_a = p_acc.tile([128, 512], F32, tag="acc", name=f"poa{qt}_{hp}")
        po_b = p_acc.tile([128, 512], F32, tag="acc", name=f"pob{qt}_{hp}")
        for kt in range(NKT):
            _pre_kt(kt)
            # scores + exp are the ACT-feeding critical path: lift their
            # priority above backlogged attnv / background work so the
            # scheduler never starves ScalarE behind lower-value matmuls.
            with tc.high_priority(offset=500):
                ps = p_mm.tile([128, 1024], F32, tag="mm", name=f"pss{qt}_{hp}_{kt}")
                nc.tensor.matmul(
                    ps[:, 0:512],
                    k_t[hp][0:64, kt * 128:(kt + 1) * 128],
                    q_t[hp][0:64, qt * 512:(qt + 1) * 512],
                    start=True, stop=True,
                )
                nc.tensor.matmul(
                    ps[:, 512:1024],
                    k_t[hp][64:128, kt * 128:(kt + 1) * 128],
                    q_t[hp][64:128, qt * 512:(qt + 1) * 512],
                    start=True, stop=True,
                )
                e = p_e.tile([128, 1024], BF16, tag="e", name=f"e{qt}_{hp}_{kt}")
                nc.scalar.activation(e[:], ps[:], EXP)
            nc.tensor.matmul(
                po_a[0:65, :], v_t[kt][:, 2 * hp, :], e[:, 0:512],
                start=(kt == 0), stop=(kt == NKT - 1),
            )
            nc.tensor.matmul(
                po_b[0:65, :], v_t[kt][:, 2 * hp + 1, :], e[:, 512:1024],
                start=(kt == 0), stop=(kt == NKT - 1),
            )
        o = p_ost.tile([128, 512], BF16, tag=f"ost{hp}", name=f"ost{qt}_{hp}")
        for half, po in ((0, po_a), (1, po_b)):
            # approx_fast mishandles nonzero base partitions -> run over
            # [0:65] (base 0) and consume row 64 only; rows 0:64 are junk.
            rcp = p_rcp.tile([128, 512], F32, tag="rcp", name=f"rcp{qt}_{hp}_{half}")
            nc.vector.reciprocal_approx_fast(out=rcp[0:65, :], in_=po[0:65, :])
            # broadcast recip into the po bank's unused partitions 64:128
            # (row 64's sums are already consumed by the reciprocal)
            nc.tensor.matmul(
                po[64:128, :], ones_sb[64:65, :], rcp[64:65, :],
                start=True, stop=True, tile_position=(64, 64),
                skip_group_check=True,
            )
            rb = p_rb.tile([64, 512], F32, tag="rb", name=f"rb{qt}_{hp}_{half}")
            nc.vector.tensor_copy(rb[:], po[64:128, :])
            nc.vector.tensor_mul(o[64 * half:64 * half + 64, :], po[0:64, :], rb[:])
        return o

    # --- emission: jit prefix, hp-outer rows, spaced background trickle ---
    ost_q = [[None] * NHP for _ in range(NQT)]

    def _pre_kt(kt):
        while bg:
            u = bg.popleft()
            if u is not None:
                u()
            break

    def spaced(units, gap):
        seq = []
        for u in units:
            seq.append(u)
            seq.extend([None] * gap)
        return seq

    emit_qk_unit(0, 0, p_mm)
    for t in range(3):
        emit_v(t, p_mm)
    qk0 = [(lambda tt=tt: emit_qk_unit(0, tt, p_bg)) for tt in range(1, NQT)]
    vs = {t: (lambda t=t: emit_v(t, p_bg)) for t in range(3, NTT)}
    # deadline order: v(t) must be EMITTED by slot t; qk0 tt_j by slot 4j
    bg.extend([vs[3], vs[4], qk0[0], vs[5], vs[6], vs[7], qk0[1], vs[8],
               vs[9], vs[10], vs[11], qk0[2], vs[12], vs[13], vs[14], vs[15]])
    bg.extend(spaced([(lambda tt=tt: emit_qk_unit(1, tt, p_bg))
                      for tt in range(NQT)], 3))
    for qt in range(NQT):
        ost_q[qt][0] = emit_attn(qt, 0)
    bg.extend(spaced([(lambda tt=tt: emit_qk_unit(2, tt, p_bg))
                      for tt in range(NQT)], 6))
    for qt in range(NQT):
        ost_q[qt][1] = emit_attn(qt, 1)
    bg.extend(spaced([(lambda tt=tt: emit_qk_unit(3, tt, p_bg))
                      for tt in range(NQT)], 6))
    for qt in range(NQT):
        ost_q[qt][2] = emit_attn(qt, 2)
    for qt in range(NQT):
        ost_q[qt][3] = emit_attn(qt, 3)
        bg.extend(spaced([
            (lambda q=qt, tl=tl, half=half:
             emit_outproj_chunk(q, ost_q[q], tl, half))
            for tl in range(4) for half in range(2)], 1))
    while bg:
        u = bg.popleft()
        if u is not None:
            u()


def _build_nc():
    nc = bacc.Bacc("TRN2", target_bir_lowering=False, debug=False, num_devices=NCORES)
    with tile.TileContext(nc) as tc, ExitStack() as ctx:
        _emit(nc, tc, ctx)
    nc.compile()
    return nc


def kernel(inputs, wq, bq, wk, bk, wv, bv, wo, bo):
    global _NC_CACHE, LAST_RESULTS
    if _NC_CACHE is None:
        _NC_CACHE = _build_nc()
    nc = _NC_CACHE

    inputs = np.asarray(inputs, dtype=np.float32)
    wq, bq, wk, bk = (np.asarray(a, dtype=np.float32) for a in (wq, bq, wk, bk))
    wv, bv, wo, bo = (np.asarray(a, dtype=np.float32) for a in (wv, bv, wo, bo))

    scale = np.float32(1.0 / np.sqrt(HD))
    wqT_full = np.ascontiguousarray(wq.T) * scale
    wkT_full = np.ascontiguousarray(wk.T)
    wvT_full = np.ascontiguousarray(wv.T)
    woT_full = np.ascontiguousarray(wo.T)

    in_maps = []
    for c in range(NCORES):
        b, g = c // 2, c % 2
        cols = slice(g * DHC, (g + 1) * DHC)
        in_maps.append({
            "xT": np.ascontiguousarray(inputs[b].T).astype(NPBF16),
            "wqT": np.ascontiguousarray(wqT_full[:, cols]).astype(NPBF16),
            "wkT": np.ascontiguousarray(wkT_full[:, cols]).astype(NPBF16),
            "wvT": np.ascontiguousarray(wvT_full[:, cols]).astype(NPBF16),
            "woT": np.ascontiguousarray(woT_full[cols, :]).astype(NPBF16),
            "bq": (bq[cols] * scale).astype(np.float32).reshape(DHC, 1),
            "bk": bk[cols].astype(np.float32).reshape(DHC, 1),
        })

    trace = bool(int(os.environ.get("BASS_KERNEL_TRACE", "0")))
    res = run_bass_kernel_spmd(nc, in_maps, core_ids=list(range(NCORES)), trace=trace)
    LAST_RESULTS = res

    fused_bias = (bo + bv @ wo.T).astype(np.float32)
    outv = np.empty((B, L, D), dtype=np.float32)
    for b in range(B):
        outv[b] = res.results[2 * b]["out"] + res.results[2 * b + 1]["out"] + fused_bias
    return outv


# revision 15
# speedup vs baseline: 1.0408x; 1.0339x over previous
"""Multi-head self-attention (B=4, L=2048, D=1024, H=16, Hd=64) on 8 TRN2 NeuronCores.

Sharding: data-parallel over batch (4) x tensor-parallel over head-groups (2).
Core c handles batch c//2 with heads [8*(c%2), 8*(c%2)+8). Each core computes a
partial out-projection over its 512 head-dims; the host sums the two partials
per batch and adds the fused bias (bo + bv @ wo.T, exact because softmax rows
sum to 1, so the v-bias passes through attention unchanged).

Per-core kernel (bf16 compute, f32 accumulation):
  - q/k feature-major [512, 2048] = w.T-slice @ x.T (bias per-partition, q
    pre-scaled by 1/sqrt(Hd) via host-scaled weights)
  - v token-major [2048, 8*65] with a ones-column per head: the attn@v matmul
    (lhsT = v_aug) then yields softmax denominators in PSUM row 64 for free
  - scores S^T[key, q] per head via K=64 matmuls, two heads packed into the
    128x128 PE array via base-partition 0/64 row tiling
  - exp on ScalarE (no max-subtraction: scores are ~N(0,1), fp32-safe)
  - normalize: DVE reciprocal of row 64 -> PE ones-broadcast -> DVE scale
  - out-projection token-major, host adds partials
"""
import os
import sys
import time
from contextlib import ExitStack

sys.path.insert(0, "/opt/trn_rl_repo")

import numpy as np
import ml_dtypes

import concourse.bass as bass
import concourse.tile as tile
from concourse import bacc, mybir
from concourse.bass_utils import run_bass_kernel_spmd

BF16 = mybir.dt.bfloat16
F32 = mybir.dt.float32
NPBF16 = ml_dtypes.bfloat16

B, L, D = 4, 2048, 1024
H, HD = 16, 64
HPC = 8            # heads per core
DHC = HPC * HD     # 512 local head-dims per core
NCORES = 8

NKD = D // 128     # 8 contraction tiles over model dim
NTT = L // 128     # 16 token tiles of 128
NQT = L // 512     # 4 query tiles of 512
NKT = L // 128     # 16 key tiles of 128
NHP = HPC // 2     # 4 head pairs

_NC_CACHE = None
LAST_RESULTS = None  # test harness introspection


def _emit(nc, tc, ctx):
    EXP = mybir.ActivationFunctionType.Exp
    ADD = mybir.AluOpType.add
    from collections import deque

    xT = nc.declare_dram_parameter("xT", [D, L], BF16, isOutput=False)
    wqT = nc.declare_dram_parameter("wqT", [D, DHC], BF16, isOutput=False)
    wkT = nc.declare_dram_parameter("wkT", [D, DHC], BF16, isOutput=False)
    wvT = nc.declare_dram_parameter("wvT", [D, DHC], BF16, isOutput=False)
    woT = nc.declare_dram_parameter("woT", [DHC, D], BF16, isOutput=False)
    bq = nc.declare_dram_parameter("bq", [DHC, 1], F32, isOutput=False)
    bk = nc.declare_dram_parameter("bk", [DHC, 1], F32, isOutput=False)
    out = nc.declare_dram_parameter("out", [L, D], F32, isOutput=True)

    p_xt = ctx.enter_context(tc.tile_pool(name="xt", bufs=NKD))
    p_wq = ctx.enter_context(tc.tile_pool(name="wq", bufs=NKD))
    p_wk = ctx.enter_context(tc.tile_pool(name="wk", bufs=NKD))
    p_wv = ctx.enter_context(tc.tile_pool(name="wv", bufs=NKD))
    p_wo = ctx.enter_context(tc.tile_pool(name="wo", bufs=4))
    p_bias = ctx.enter_context(tc.tile_pool(name="bias", bufs=2 * NHP + 1))
    p_q = ctx.enter_context(tc.tile_pool(name="q", bufs=NHP))
    p_k = ctx.enter_context(tc.tile_pool(name="k", bufs=NHP))
    p_v = ctx.enter_context(tc.tile_pool(name="v", bufs=NTT))
    p_e = ctx.enter_context(tc.tile_pool(name="e", bufs=8))
    p_ost = ctx.enter_context(tc.tile_pool(name="ost", bufs=NQT + 1))
    p_rcp = ctx.enter_context(tc.tile_pool(name="rcp", bufs=2))
    p_rb = ctx.enter_context(tc.tile_pool(name="rb", bufs=2))
    p_outst = ctx.enter_context(tc.tile_pool(name="outst", bufs=3))
    # PSUM: scores 2x[128,1024] (4 banks) + attnv accum 2x[128,512] (2 banks)
    # + one background ring [128,1024] (2 banks) = 8 banks exactly.
    p_mm = ctx.enter_context(tc.tile_pool(name="pmm", bufs=2, space="PSUM"))
    p_acc = ctx.enter_context(tc.tile_pool(name="pacc", bufs=2, space="PSUM"))
    p_bg = ctx.enter_context(tc.tile_pool(name="pbg", bufs=1, space="PSUM"))

    # --- weight / input DMAs ---
    xt = [p_xt.tile([128, L], BF16, tag="xt", name=f"xt{i}") for i in range(NKD)]
    wqs = [p_wq.tile([128, DHC], BF16, tag="wq", name=f"wqs{i}") for i in range(NKD)]
    wks = [p_wk.tile([128, DHC], BF16, tag="wk", name=f"wks{i}") for i in range(NKD)]
    wvs = [p_wv.tile([128, DHC], BF16, tag="wv", name=f"wvs{i}") for i in range(NKD)]
    wos = [p_wo.tile([128, D], BF16, tag="wo", name=f"wos{i}") for i in range(4)]
    for kd in range(NKD):
        nc.sync.dma_start(xt[kd][:], xT[kd * 128:(kd + 1) * 128, :])
        nc.sync.dma_start(wqs[kd][:], wqT[kd * 128:(kd + 1) * 128, :])
        nc.sync.dma_start(wks[kd][:], wkT[kd * 128:(kd + 1) * 128, :])
    for kd in range(NKD):
        nc.gpsimd.dma_start(wvs[kd][:], wvT[kd * 128:(kd + 1) * 128, :])
    for j in range(4):
        nc.gpsimd.dma_start(wos[j][:], woT[j * 128:(j + 1) * 128, :])
    bqt, bkt = [], []
    for hp in range(NHP):
        tq = p_bias.tile([128, 1], F32, tag="bias")
        tk = p_bias.tile([128, 1], F32, tag="bias")
        nc.gpsimd.dma_start(tq[:], bq[hp * 128:(hp + 1) * 128, :])
        nc.gpsimd.dma_start(tk[:], bk[hp * 128:(hp + 1) * 128, :])
        bqt.append(tq)
        bkt.append(tk)
    ones_sb = p_bias.tile([128, 64], F32, tag="bias")
    nc.vector.memset(ones_sb[:], 1.0)

    q_t, k_t = [None] * NHP, [None] * NHP
    v_t = [None] * NTT
    outst_q = {}

    def emit_qk_unit(hp, tt, pool):
        if q_t[hp] is None:
            q_t[hp] = p_q.tile([128, L], BF16, tag="q", name=f"qT{hp}")
            k_t[hp] = p_k.tile([128, L], BF16, tag="k", name=f"kT{hp}")
        qt_, kt_ = q_t[hp], k_t[hp]
        ps = pool.tile([128, 1024], F32, tag=pool.name[1:], name=f"psqk{hp}_{tt}")
        for kd in range(NKD):
            nc.tensor.matmul(
                ps[:, 0:512], wqs[kd][:, hp * 128:(hp + 1) * 128],
                xt[kd][:, tt * 512:(tt + 1) * 512],
                start=(kd == 0), stop=(kd == NKD - 1),
            )
        for kd in range(NKD):
            nc.tensor.matmul(
                ps[:, 512:1024], wks[kd][:, hp * 128:(hp + 1) * 128],
                xt[kd][:, tt * 512:(tt + 1) * 512],
                start=(kd == 0), stop=(kd == NKD - 1),
            )
        nc.vector.tensor_scalar(
            qt_[:, tt * 512:(tt + 1) * 512], ps[:, 0:512], bqt[hp][:], None, ADD)
        nc.vector.tensor_scalar(
            kt_[:, tt * 512:(tt + 1) * 512], ps[:, 512:1024], bkt[hp][:], None, ADD)

    def emit_v(t, pool):
        ps = pool.tile([128, 1024], F32, tag=pool.name[1:], name=f"psv{t}")
        for kd in range(NKD):
            nc.tensor.matmul(
                ps[:, 0:512], xt[kd][:, t * 128:(t + 1) * 128], wvs[kd][:],
                start=(kd == 0), stop=(kd == NKD - 1),
            )
        vt = p_v.tile([128, HPC, HD + 1], BF16, tag="v", name=f"v{t}")
        nc.vector.memset(vt[:, :, HD:HD + 1], 1.0)
        nc.vector.tensor_copy(
            vt[:, :, 0:HD], ps[:, 0:512].rearrange("p (h d) -> p h d", h=HPC))
        v_t[t] = vt

    def emit_outproj_chunk(qt, ost, tl, half):
        key = (qt, tl)
        if key not in outst_q:
            outst_q[key] = p_outst.tile(
                [128, 1024], F32, tag="outst", name=f"outst{qt}_{tl}")
        outst = outst_q[key]
        ps_o = p_bg.tile([128, 1024], F32, tag="bg", name=f"pso{qt}_{tl}_{half}")
        for j in range(NHP):
            nc.tensor.matmul(
                ps_o[:, 0:512],
                ost[j][:, tl * 128:(tl + 1) * 128],
                wos[j][:, half * 512:(half + 1) * 512],
                start=(j == 0), stop=(j == NHP - 1),
            )
        nc.vector.tensor_copy(outst[:, half * 512:(half + 1) * 512], ps_o[:, 0:512])
        if half == 1:
            t = qt * 4 + tl
            nc.gpsimd.dma_start(out[t * 128:(t + 1) * 128, :], outst[:])

    bg = deque()

    def _pre_kt(kt):
        if bg:
            bg.popleft()()

    def emit_attn(qt, hp):
        po_a = p_acc.tile([128, 512], F32, tag="acc", name=f"poa{qt}_{hp}")
        po_b = p_acc.tile([128, 512], F32, tag="acc", name=f"pob{qt}_{hp}")
        for kt in range(NKT):
            _pre_kt(kt)
            # scores + exp are the ACT-feeding critical path: lift their
            # priority above backlogged attnv / background work so the
            # scheduler never starves ScalarE behind lower-value matmuls.
            with tc.high_priority(offset=300):
                ps = p_mm.tile([128, 1024], F32, tag="mm", name=f"pss{qt}_{hp}_{kt}")
                nc.tensor.matmul(
                    ps[:, 0:512],
                    k_t[hp][0:64, kt * 128:(kt + 1) * 128],
                    q_t[hp][0:64, qt * 512:(qt + 1) * 512],
                    start=True, stop=True,
                )
                nc.tensor.matmul(
                    ps[:, 512:1024],
                    k_t[hp][64:128, kt * 128:(kt + 1) * 128],
                    q_t[hp][64:128, qt * 512:(qt + 1) * 512],
                    start=True, stop=True,
                )
                e = p_e.tile([128, 1024], BF16, tag="e", name=f"e{qt}_{hp}_{kt}")
                nc.scalar.activation(e[:], ps[:], EXP)
            nc.tensor.matmul(
                po_a[0:65, :], v_t[kt][:, 2 * hp, :], e[:, 0:512],
                start=(kt == 0), stop=(kt == NKT - 1),
            )
            nc.tensor.matmul(
                po_b[0:65, :], v_t[kt][:, 2 * hp + 1, :], e[:, 512:1024],
                start=(kt == 0), stop=(kt == NKT - 1),
            )
        o = p_ost.tile([128, 512], BF16, tag=f"ost{hp}", name=f"ost{qt}_{hp}")
        for half, po in ((0, po_a), (1, po_b)):
            # approx_fast mishandles nonzero base partitions -> run over
            # [0:65] (base 0) and consume row 64 only; rows 0:64 are junk.
            rcp = p_rcp.tile([128, 512], F32, tag="rcp", name=f"rcp{qt}_{hp}_{half}")
            nc.vector.reciprocal_approx_fast(out=rcp[0:65, :], in_=po[0:65, :])
            # broadcast recip into the po bank's unused partitions 64:128
            # (row 64's sums are already consumed by the reciprocal)
            nc.tensor.matmul(
                po[64:128, :], ones_sb[64:65, :], rcp[64:65, :],
                start=True, stop=True, tile_position=(64, 64),
                skip_group_check=True,
            )
            rb = p_rb.tile([64, 512], F32, tag="rb", name=f"rb{qt}_{hp}_{half}")
            nc.vector.tensor_copy(rb[:], po[64:128, :])
            nc.vector.tensor_mul(o[64 * half:64 * half + 64, :], po[0:64, :], rb[:])
        return o

    # --- emission: jit prefix, hp-outer rows, spaced background trickle ---
    ost_q = [[None] * NHP for _ in range(NQT)]

    def _pre_kt(kt):
        while bg:
            u = bg.popleft()
            if u is not None:
                u()
            break

    def spaced(units, gap):
        seq = []
        for u in units:
            seq.append(u)
            seq.extend([None] * gap)
        return seq

    emit_qk_unit(0, 0, p_mm)
    for t in range(3):
        emit_v(t, p_mm)
    qk0 = [(lambda tt=tt: emit_qk_unit(0, tt, p_bg)) for tt in range(1, NQT)]
    vs = {t: (lambda t=t: emit_v(t, p_bg)) for t in range(3, NTT)}
    # deadline order: v(t) must be EMITTED by slot t; qk0 tt_j by slot 4j
    bg.extend([vs[3], vs[4], qk0[0], vs[5], vs[6], vs[7], qk0[1], vs[8],
               vs[9], vs[10], vs[11], qk0[2], vs[12], vs[13], vs[14], vs[15]])
    bg.extend(spaced([(lambda tt=tt: emit_qk_unit(1, tt, p_bg))
                      for tt in range(NQT)], 3))
    for qt in range(NQT):
        ost_q[qt][0] = emit_attn(qt, 0)
    bg.extend(spaced([(lambda tt=tt: emit_qk_unit(2, tt, p_bg))
                      for tt in range(NQT)], 6))
    for qt in range(NQT):
        ost_q[qt][1] = emit_attn(qt, 1)
    bg.extend(spaced([(lambda tt=tt: emit_qk_unit(3, tt, p_bg))
                      for tt in range(NQT)], 6))
    for qt in range(NQT):
        ost_q[qt][2] = emit_attn(qt, 2)
    for qt in range(NQT):
        ost_q[qt][3] = emit_attn(qt, 3)
        if qt < NQT - 1:
            bg.extend(spaced([
                (lambda q=qt, tl=tl, half=half:
                 emit_outproj_chunk(q, ost_q[q], tl, half))
                for tl in range(4) for half in range(2)], 1))
    while bg:
        u = bg.popleft()
        if u is not None:
            u()
    # final query tile: scores ring is idle now, use its big slots
    for tl in range(4):
        qt = NQT - 1
        outst = p_outst.tile([128, 1024], F32, tag="outst", name=f"outstF_{tl}")
        ps_o = p_mm.tile([128, 1024], F32, tag="mm", name=f"psoF_{tl}")
        for half in range(2):
            for j in range(NHP):
                nc.tensor.matmul(
                    ps_o[:, half * 512:(half + 1) * 512],
                    ost_q[qt][j][:, tl * 128:(tl + 1) * 128],
                    wos[j][:, half * 512:(half + 1) * 512],
                    start=(j == 0), stop=(j == NHP - 1),
                )
        nc.vector.tensor_copy(outst[:], ps_o[:])
        t = qt * 4 + tl
        nc.gpsimd.dma_start(out[t * 128:(t + 1) * 128, :], outst[:])


def _build_nc():
    nc = bacc.Bacc("TRN2", target_bir_lowering=False, debug=False, num_devices=NCORES)
    with tile.TileContext(nc) as tc, ExitStack() as ctx:
        _emit(nc, tc, ctx)
    nc.compile()
    return nc


def kernel(inputs, wq, bq, wk, bk, wv, bv, wo, bo):
    global _NC_CACHE, LAST_RESULTS
    if _NC_CACHE is None:
        _NC_CACHE = _build_nc()
    nc = _NC_CACHE

    inputs = np.asarray(inputs, dtype=np.float32)
    wq, bq, wk, bk = (np.asarray(a, dtype=np.float32) for a in (wq, bq, wk, bk))
    wv, bv, wo, bo = (np.asarray(a, dtype=np.float32) for a in (wv, bv, wo, bo))

    scale = np.float32(1.0 / np.sqrt(HD))
    wqT_full = np.ascontiguousarray(wq.T) * scale
    wkT_full = np.ascontiguousarray(wk.T)
    wvT_full = np.ascontiguousarray(wv.T)
    woT_full = np.ascontiguousarray(wo.T)

    in_maps = []
    for c in range(NCORES):
        b, g = c // 2, c % 2
        cols = slice(g * DHC, (g + 1) * DHC)
        in_maps.append({
            "xT": np.ascontiguousarray(inputs[b].T).astype(NPBF16),
            "wqT": np.ascontiguousarray(wqT_full[:, cols]).astype(NPBF16),
            "wkT": np.ascontiguousarray(wkT_full[:, cols]).astype(NPBF16),
            "wvT": np.ascontiguousarray(wvT_full[:, cols]).astype(NPBF16),
            "woT": np.ascontiguousarray(woT_full[cols, :]).astype(NPBF16),
            "bq": (bq[cols] * scale).astype(np.float32).reshape(DHC, 1),
            "bk": bk[cols].astype(np.float32).reshape(DHC, 1),
        })

    trace = bool(int(os.environ.get("BASS_KERNEL_TRACE", "0")))
    res = run_bass_kernel_spmd(nc, in_maps, core_ids=list(range(NCORES)), trace=trace)
    LAST_RESULTS = res

    fused_bias = (bo + bv @ wo.T).astype(np.float32)
    outv = np.empty((B, L, D), dtype=np.float32)
    for b in range(B):
        outv[b] = res.results[2 * b]["out"] + res.results[2 * b + 1]["out"] + fused_bias
    return outv


# revision 16
# speedup vs baseline: 1.0582x; 1.0167x over previous
"""Multi-head self-attention (B=4, L=2048, D=1024, H=16, Hd=64) on 8 TRN2 NeuronCores.

Sharding: data-parallel over batch (4) x tensor-parallel over head-groups (2).
Core c handles batch c//2 with heads [8*(c%2), 8*(c%2)+8). Each core computes a
partial out-projection over its 512 head-dims; the host sums the two partials
per batch and adds the fused bias (bo + bv @ wo.T, exact because softmax rows
sum to 1, so the v-bias passes through attention unchanged).

Per-core kernel (bf16 compute, f32 accumulation):
  - q/k feature-major [512, 2048] = w.T-slice @ x.T (bias per-partition, q
    pre-scaled by 1/sqrt(Hd) via host-scaled weights)
  - v token-major [2048, 8*65] with a ones-column per head: the attn@v matmul
    (lhsT = v_aug) then yields softmax denominators in PSUM row 64 for free
  - scores S^T[key, q] per head via K=64 matmuls, two heads packed into the
    128x128 PE array via base-partition 0/64 row tiling
  - exp on ScalarE (no max-subtraction: scores are ~N(0,1), fp32-safe)
  - normalize: DVE reciprocal of row 64 -> PE ones-broadcast -> DVE scale
  - out-projection token-major, host adds partials
"""
import os
import sys
import time
from contextlib import ExitStack

sys.path.insert(0, "/opt/trn_rl_repo")

import numpy as np
import ml_dtypes

import concourse.bass as bass
import concourse.tile as tile
from concourse import bacc, mybir
from concourse.bass_utils import run_bass_kernel_spmd

BF16 = mybir.dt.bfloat16
F32 = mybir.dt.float32
NPBF16 = ml_dtypes.bfloat16

B, L, D = 4, 2048, 1024
H, HD = 16, 64
HPC = 8            # heads per core
DHC = HPC * HD     # 512 local head-dims per core
NCORES = 8

NKD = D // 128     # 8 contraction tiles over model dim
NTT = L // 128     # 16 token tiles of 128
NQT = L // 512     # 4 query tiles of 512
NKT = L // 128     # 16 key tiles of 128
NHP = HPC // 2     # 4 head pairs

_NC_CACHE = None
LAST_RESULTS = None  # test harness introspection


def _emit(nc, tc, ctx):
    EXP = mybir.ActivationFunctionType.Exp
    ADD = mybir.AluOpType.add
    from collections import deque

    xT = nc.declare_dram_parameter("xT", [D, L], BF16, isOutput=False)
    wqT = nc.declare_dram_parameter("wqT", [D, DHC], BF16, isOutput=False)
    wkT = nc.declare_dram_parameter("wkT", [D, DHC], BF16, isOutput=False)
    wvT = nc.declare_dram_parameter("wvT", [D, DHC], BF16, isOutput=False)
    woT = nc.declare_dram_parameter("woT", [DHC, D], BF16, isOutput=False)
    bq = nc.declare_dram_parameter("bq", [DHC, 1], F32, isOutput=False)
    bk = nc.declare_dram_parameter("bk", [DHC, 1], F32, isOutput=False)
    out = nc.declare_dram_parameter("out", [L, D], F32, isOutput=True)

    p_xt = ctx.enter_context(tc.tile_pool(name="xt", bufs=NKD))
    p_wq = ctx.enter_context(tc.tile_pool(name="wq", bufs=NKD))
    p_wk = ctx.enter_context(tc.tile_pool(name="wk", bufs=NKD))
    p_wv = ctx.enter_context(tc.tile_pool(name="wv", bufs=NKD))
    p_wo = ctx.enter_context(tc.tile_pool(name="wo", bufs=4))
    p_bias = ctx.enter_context(tc.tile_pool(name="bias", bufs=2 * NHP + 1))
    p_q = ctx.enter_context(tc.tile_pool(name="q", bufs=NHP))
    p_k = ctx.enter_context(tc.tile_pool(name="k", bufs=NHP))
    p_v = ctx.enter_context(tc.tile_pool(name="v", bufs=NTT))
    p_e = ctx.enter_context(tc.tile_pool(name="e", bufs=8))
    p_ost = ctx.enter_context(tc.tile_pool(name="ost", bufs=NQT + 1))
    p_rcp = ctx.enter_context(tc.tile_pool(name="rcp", bufs=2))
    p_rb = ctx.enter_context(tc.tile_pool(name="rb", bufs=2))
    p_outst = ctx.enter_context(tc.tile_pool(name="outst", bufs=3))
    # PSUM: scores 2x[128,1024] (4 banks) + attnv accum 2x[128,512] (2 banks)
    # + one background ring [128,1024] (2 banks) = 8 banks exactly.
    p_mm = ctx.enter_context(tc.tile_pool(name="pmm", bufs=2, space="PSUM"))
    p_acc = ctx.enter_context(tc.tile_pool(name="pacc", bufs=2, space="PSUM"))
    p_bg = ctx.enter_context(tc.tile_pool(name="pbg", bufs=1, space="PSUM"))

    # --- weight / input DMAs ---
    xt = [p_xt.tile([128, L], BF16, tag="xt", name=f"xt{i}") for i in range(NKD)]
    wqs = [p_wq.tile([128, DHC], BF16, tag="wq", name=f"wqs{i}") for i in range(NKD)]
    wks = [p_wk.tile([128, DHC], BF16, tag="wk", name=f"wks{i}") for i in range(NKD)]
    wvs = [p_wv.tile([128, DHC], BF16, tag="wv", name=f"wvs{i}") for i in range(NKD)]
    wos = [p_wo.tile([128, D], BF16, tag="wo", name=f"wos{i}") for i in range(4)]
    # first-needed chunks first: qk0_tt0 needs xt[:, 0:512] + w[:, 0:128]
    for kd in range(NKD):
        nc.sync.dma_start(xt[kd][:, 0:512], xT[kd * 128:(kd + 1) * 128, 0:512])
        nc.sync.dma_start(wqs[kd][:, 0:128], wqT[kd * 128:(kd + 1) * 128, 0:128])
        nc.sync.dma_start(wks[kd][:, 0:128], wkT[kd * 128:(kd + 1) * 128, 0:128])
    for kd in range(NKD):
        nc.sync.dma_start(xt[kd][:, 512:2048], xT[kd * 128:(kd + 1) * 128, 512:2048])
        nc.gpsimd.dma_start(wvs[kd][:], wvT[kd * 128:(kd + 1) * 128, :])
    for kd in range(NKD):
        nc.sync.dma_start(wqs[kd][:, 128:512], wqT[kd * 128:(kd + 1) * 128, 128:512])
        nc.sync.dma_start(wks[kd][:, 128:512], wkT[kd * 128:(kd + 1) * 128, 128:512])
    for j in range(4):
        nc.gpsimd.dma_start(wos[j][:], woT[j * 128:(j + 1) * 128, :])
    bqt, bkt = [], []
    for hp in range(NHP):
        tq = p_bias.tile([128, 1], F32, tag="bias")
        tk = p_bias.tile([128, 1], F32, tag="bias")
        nc.gpsimd.dma_start(tq[:], bq[hp * 128:(hp + 1) * 128, :])
        nc.gpsimd.dma_start(tk[:], bk[hp * 128:(hp + 1) * 128, :])
        bqt.append(tq)
        bkt.append(tk)
    ones_sb = p_bias.tile([128, 64], F32, tag="bias")
    nc.vector.memset(ones_sb[:], 1.0)

    q_t, k_t = [None] * NHP, [None] * NHP
    v_t = [None] * NTT
    outst_q = {}

    def emit_qk_unit(hp, tt, pool):
        if q_t[hp] is None:
            q_t[hp] = p_q.tile([128, L], BF16, tag="q", name=f"qT{hp}")
            k_t[hp] = p_k.tile([128, L], BF16, tag="k", name=f"kT{hp}")
        qt_, kt_ = q_t[hp], k_t[hp]
        ps = pool.tile([128, 1024], F32, tag=pool.name[1:], name=f"psqk{hp}_{tt}")
        for kd in range(NKD):
            nc.tensor.matmul(
                ps[:, 0:512], wqs[kd][:, hp * 128:(hp + 1) * 128],
                xt[kd][:, tt * 512:(tt + 1) * 512],
                start=(kd == 0), stop=(kd == NKD - 1),
            )
        for kd in range(NKD):
            nc.tensor.matmul(
                ps[:, 512:1024], wks[kd][:, hp * 128:(hp + 1) * 128],
                xt[kd][:, tt * 512:(tt + 1) * 512],
                start=(kd == 0), stop=(kd == NKD - 1),
            )
        nc.vector.tensor_scalar(
            qt_[:, tt * 512:(tt + 1) * 512], ps[:, 0:512], bqt[hp][:], None, ADD)
        nc.vector.tensor_scalar(
            kt_[:, tt * 512:(tt + 1) * 512], ps[:, 512:1024], bkt[hp][:], None, ADD)

    def emit_v(t, pool):
        ps = pool.tile([128, 1024], F32, tag=pool.name[1:], name=f"psv{t}")
        for kd in range(NKD):
            nc.tensor.matmul(
                ps[:, 0:512], xt[kd][:, t * 128:(t + 1) * 128], wvs[kd][:],
                start=(kd == 0), stop=(kd == NKD - 1),
            )
        vt = p_v.tile([128, HPC, HD + 1], BF16, tag="v", name=f"v{t}")
        nc.vector.memset(vt[:, :, HD:HD + 1], 1.0)
        nc.vector.tensor_copy(
            vt[:, :, 0:HD], ps[:, 0:512].rearrange("p (h d) -> p h d", h=HPC))
        v_t[t] = vt

    def emit_outproj_chunk(qt, ost, tl, half):
        key = (qt, tl)
        if key not in outst_q:
            outst_q[key] = p_outst.tile(
                [128, 1024], F32, tag="outst", name=f"outst{qt}_{tl}")
        outst = outst_q[key]
        ps_o = p_bg.tile([128, 1024], F32, tag="bg", name=f"pso{qt}_{tl}_{half}")
        for j in range(NHP):
            nc.tensor.matmul(
                ps_o[:, 0:512],
                ost[j][:, tl * 128:(tl + 1) * 128],
                wos[j][:, half * 512:(half + 1) * 512],
                start=(j == 0), stop=(j == NHP - 1),
            )
        nc.vector.tensor_copy(outst[:, half * 512:(half + 1) * 512], ps_o[:, 0:512])
        if half == 1:
            t = qt * 4 + tl
            nc.gpsimd.dma_start(out[t * 128:(t + 1) * 128, :], outst[:])

    bg = deque()

    def _pre_kt(kt):
        if bg:
            bg.popleft()()

    def emit_attn(qt, hp):
        po_a = p_acc.tile([128, 512], F32, tag="acc", name=f"poa{qt}_{hp}")
        po_b = p_acc.tile([128, 512], F32, tag="acc", name=f"pob{qt}_{hp}")
        for kt in range(NKT):
            _pre_kt(kt)
            # scores + exp are the ACT-feeding critical path: lift their
            # priority above backlogged attnv / background work so the
            # scheduler never starves ScalarE behind lower-value matmuls.
            with tc.high_priority(offset=300):
                ps = p_mm.tile([128, 1024], F32, tag="mm", name=f"pss{qt}_{hp}_{kt}")
                nc.tensor.matmul(
                    ps[:, 0:512],
                    k_t[hp][0:64, kt * 128:(kt + 1) * 128],
                    q_t[hp][0:64, qt * 512:(qt + 1) * 512],
                    start=True, stop=True,
                )
                nc.tensor.matmul(
                    ps[:, 512:1024],
                    k_t[hp][64:128, kt * 128:(kt + 1) * 128],
                    q_t[hp][64:128, qt * 512:(qt + 1) * 512],
                    start=True, stop=True,
                )
                e = p_e.tile([128, 1024], BF16, tag="e", name=f"e{qt}_{hp}_{kt}")
                nc.scalar.activation(e[:], ps[:], EXP)
            nc.tensor.matmul(
                po_a[0:65, :], v_t[kt][:, 2 * hp, :], e[:, 0:512],
                start=(kt == 0), stop=(kt == NKT - 1),
            )
            nc.tensor.matmul(
                po_b[0:65, :], v_t[kt][:, 2 * hp + 1, :], e[:, 512:1024],
                start=(kt == 0), stop=(kt == NKT - 1),
            )
        o = p_ost.tile([128, 512], BF16, tag=f"ost{hp}", name=f"ost{qt}_{hp}")
        for half, po in ((0, po_a), (1, po_b)):
            # approx_fast mishandles nonzero base partitions -> run over
            # [0:65] (base 0) and consume row 64 only; rows 0:64 are junk.
            rcp = p_rcp.tile([128, 512], F32, tag="rcp", name=f"rcp{qt}_{hp}_{half}")
            nc.vector.reciprocal_approx_fast(out=rcp[0:65, :], in_=po[0:65, :])
            # broadcast recip into the po bank's unused partitions 64:128
            # (row 64's sums are already consumed by the reciprocal)
            nc.tensor.matmul(
                po[64:128, :], ones_sb[64:65, :], rcp[64:65, :],
                start=True, stop=True, tile_position=(64, 64),
                skip_group_check=True,
            )
            rb = p_rb.tile([64, 512], F32, tag="rb", name=f"rb{qt}_{hp}_{half}")
            nc.vector.tensor_copy(rb[:], po[64:128, :])
            nc.vector.tensor_mul(o[64 * half:64 * half + 64, :], po[0:64, :], rb[:])
        return o

    # --- emission: jit prefix, hp-outer rows, spaced background trickle ---
    ost_q = [[None] * NHP for _ in range(NQT)]

    def _pre_kt(kt):
        while bg:
            u = bg.popleft()
            if u is not None:
                u()
            break

    def spaced(units, gap):
        seq = []
        for u in units:
            seq.append(u)
            seq.extend([None] * gap)
        return seq

    emit_qk_unit(0, 0, p_mm)
    for t in range(3):
        emit_v(t, p_mm)
    qk0 = [(lambda tt=tt: emit_qk_unit(0, tt, p_bg)) for tt in range(1, NQT)]
    vs = {t: (lambda t=t: emit_v(t, p_bg)) for t in range(3, NTT)}
    # deadline order: v(t) must be EMITTED by slot t; qk0 tt_j by slot 4j
    bg.extend([vs[3], vs[4], qk0[0], vs[5], vs[6], vs[7], qk0[1], vs[8],
               vs[9], vs[10], vs[11], qk0[2], vs[12], vs[13], vs[14], vs[15]])
    bg.extend(spaced([(lambda tt=tt: emit_qk_unit(1, tt, p_bg))
                      for tt in range(NQT)], 3))
    for qt in range(NQT):
        ost_q[qt][0] = emit_attn(qt, 0)
    bg.extend(spaced([(lambda tt=tt: emit_qk_unit(2, tt, p_bg))
                      for tt in range(NQT)], 6))
    for qt in range(NQT):
        ost_q[qt][1] = emit_attn(qt, 1)
    bg.extend(spaced([(lambda tt=tt: emit_qk_unit(3, tt, p_bg))
                      for tt in range(NQT)], 6))
    for qt in range(NQT):
        ost_q[qt][2] = emit_attn(qt, 2)
    for qt in range(NQT):
        ost_q[qt][3] = emit_attn(qt, 3)
        if qt < NQT - 1:
            bg.extend(spaced([
                (lambda q=qt, tl=tl, half=half:
                 emit_outproj_chunk(q, ost_q[q], tl, half))
                for tl in range(4) for half in range(2)], 1))
    while bg:
        u = bg.popleft()
        if u is not None:
            u()
    # final query tile: scores ring is idle now, use its big slots
    for tl in range(4):
        qt = NQT - 1
        outst = p_outst.tile([128, 1024], F32, tag="outst", name=f"outstF_{tl}")
        ps_o = p_mm.tile([128, 1024], F32, tag="mm", name=f"psoF_{tl}")
        for half in range(2):
            for j in range(NHP):
                nc.tensor.matmul(
                    ps_o[:, half * 512:(half + 1) * 512],
                    ost_q[qt][j][:, tl * 128:(tl + 1) * 128],
                    wos[j][:, half * 512:(half + 1) * 512],
                    start=(j == 0), stop=(j == NHP - 1),
                )
        nc.vector.tensor_copy(outst[:], ps_o[:])
        t = qt * 4 + tl
        nc.gpsimd.dma_start(out[t * 128:(t + 1) * 128, :], outst[:])


def _build_nc():
    nc = bacc.Bacc("TRN2", target_bir_lowering=False, debug=False, num_devices=NCORES)
    with tile.TileContext(nc) as tc, ExitStack() as ctx:
        _emit(nc, tc, ctx)
    nc.compile()
    return nc


def kernel(inputs, wq, bq, wk, bk, wv, bv, wo, bo):
    global _NC_CACHE, LAST_RESULTS
    if _NC_CACHE is None:
        _NC_CACHE = _build_nc()
    nc = _NC_CACHE

    inputs = np.asarray(inputs, dtype=np.float32)
    wq, bq, wk, bk = (np.asarray(a, dtype=np.float32) for a in (wq, bq, wk, bk))
    wv, bv, wo, bo = (np.asarray(a, dtype=np.float32) for a in (wv, bv, wo, bo))

    scale = np.float32(1.0 / np.sqrt(HD))
    wqT_full = np.ascontiguousarray(wq.T) * scale
    wkT_full = np.ascontiguousarray(wk.T)
    wvT_full = np.ascontiguousarray(wv.T)
    woT_full = np.ascontiguousarray(wo.T)

    in_maps = []
    for c in range(NCORES):
        b, g = c // 2, c % 2
        cols = slice(g * DHC, (g + 1) * DHC)
        in_maps.append({
            "xT": np.ascontiguousarray(inputs[b].T).astype(NPBF16),
            "wqT": np.ascontiguousarray(wqT_full[:, cols]).astype(NPBF16),
            "wkT": np.ascontiguousarray(wkT_full[:, cols]).astype(NPBF16),
            "wvT": np.ascontiguousarray(wvT_full[:, cols]).astype(NPBF16),
            "woT": np.ascontiguousarray(woT_full[cols, :]).astype(NPBF16),
            "bq": (bq[cols] * scale).astype(np.float32).reshape(DHC, 1),
            "bk": bk[cols].astype(np.float32).reshape(DHC, 1),
        })

    trace = bool(int(os.environ.get("BASS_KERNEL_TRACE", "0")))
    res = run_bass_kernel_spmd(nc, in_maps, core_ids=list(range(NCORES)), trace=trace)
    LAST_RESULTS = res

    fused_bias = (bo + bv @ wo.T).astype(np.float32)
    outv = np.empty((B, L, D), dtype=np.float32)
    for b in range(B):
        outv[b] = res.results[2 * b]["out"] + res.results[2 * b + 1]["out"] + fused_bias
    return outv


# revision 19
# speedup vs baseline: 1.0688x; 1.0100x over previous
"""Multi-head self-attention (B=4, L=2048, D=1024, H=16, Hd=64) on 8 TRN2 NeuronCores.

Sharding: data-parallel over batch (4) x tensor-parallel over head-groups (2).
Core c handles batch c//2 with heads [8*(c%2), 8*(c%2)+8). Each core computes a
partial out-projection over its 512 head-dims; the host sums the two partials
per batch and adds the fused bias (bo + bv @ wo.T, exact because softmax rows
sum to 1, so the v-bias passes through attention unchanged).

Per-core kernel (bf16 compute, f32 accumulation):
  - q/k feature-major [512, 2048] = w.T-slice @ x.T (bias per-partition, q
    pre-scaled by 1/sqrt(Hd) via host-scaled weights)
  - v token-major [2048, 8*65] with a ones-column per head: the attn@v matmul
    (lhsT = v_aug) then yields softmax denominators in PSUM row 64 for free
  - scores S^T[key, q] per head via K=64 matmuls, two heads packed into the
    128x128 PE array via base-partition 0/64 row tiling
  - exp on ScalarE (no max-subtraction: scores are ~N(0,1), fp32-safe)
  - normalize: DVE reciprocal of row 64 -> PE ones-broadcast -> DVE scale
  - out-projection token-major, host adds partials
"""
import os
import sys
import time
from contextlib import ExitStack

sys.path.insert(0, "/opt/trn_rl_repo")

import numpy as np
import ml_dtypes

import concourse.bass as bass
import concourse.tile as tile
from concourse import bacc, mybir
from concourse.bass_utils import run_bass_kernel_spmd

BF16 = mybir.dt.bfloat16
F32 = mybir.dt.float32
NPBF16 = ml_dtypes.bfloat16

B, L, D = 4, 2048, 1024
H, HD = 16, 64
HPC = 8            # heads per core
DHC = HPC * HD     # 512 local head-dims per core
NCORES = 8

NKD = D // 128     # 8 contraction tiles over model dim
NTT = L // 128     # 16 token tiles of 128
NQT = L // 512     # 4 query tiles of 512
NKT = L // 128     # 16 key tiles of 128
NHP = HPC // 2     # 4 head pairs

_NC_CACHE = None
LAST_RESULTS = None  # test harness introspection


def _emit(nc, tc, ctx):
    EXP = mybir.ActivationFunctionType.Exp
    ADD = mybir.AluOpType.add
    from collections import deque

    xT = nc.declare_dram_parameter("xT", [D, L], BF16, isOutput=False)
    wqT = nc.declare_dram_parameter("wqT", [D, DHC], BF16, isOutput=False)
    wkT = nc.declare_dram_parameter("wkT", [D, DHC], BF16, isOutput=False)
    wvT = nc.declare_dram_parameter("wvT", [D, DHC], BF16, isOutput=False)
    woT = nc.declare_dram_parameter("woT", [DHC, D], BF16, isOutput=False)
    bq = nc.declare_dram_parameter("bq", [DHC, 1], F32, isOutput=False)
    bk = nc.declare_dram_parameter("bk", [DHC, 1], F32, isOutput=False)
    out = nc.declare_dram_parameter("out", [L, D], F32, isOutput=True)

    p_xt = ctx.enter_context(tc.tile_pool(name="xt", bufs=NKD))
    p_wq = ctx.enter_context(tc.tile_pool(name="wq", bufs=NKD))
    p_wk = ctx.enter_context(tc.tile_pool(name="wk", bufs=NKD))
    p_wv = ctx.enter_context(tc.tile_pool(name="wv", bufs=NKD))
    p_wo = ctx.enter_context(tc.tile_pool(name="wo", bufs=4))
    p_bias = ctx.enter_context(tc.tile_pool(name="bias", bufs=2 * NHP + 1))
    p_q = ctx.enter_context(tc.tile_pool(name="q", bufs=NHP))
    p_k = ctx.enter_context(tc.tile_pool(name="k", bufs=NHP))
    p_v = ctx.enter_context(tc.tile_pool(name="v", bufs=NTT))
    p_e = ctx.enter_context(tc.tile_pool(name="e", bufs=8))
    p_ost = ctx.enter_context(tc.tile_pool(name="ost", bufs=NQT + 1))
    p_rcp = ctx.enter_context(tc.tile_pool(name="rcp", bufs=2))
    p_rb = ctx.enter_context(tc.tile_pool(name="rb", bufs=2))
    p_outst = ctx.enter_context(tc.tile_pool(name="outst", bufs=3))
    # PSUM: scores 2x[128,1024] (4 banks) + attnv accum 2x[128,512] (2 banks)
    # + one background ring [128,1024] (2 banks) = 8 banks exactly.
    p_mm = ctx.enter_context(tc.tile_pool(name="pmm", bufs=2, space="PSUM"))
    p_acc = ctx.enter_context(tc.tile_pool(name="pacc", bufs=3, space="PSUM"))
    p_bg = ctx.enter_context(tc.tile_pool(name="pbg", bufs=1, space="PSUM"))

    # --- weight / input DMAs ---
    xt = [p_xt.tile([128, L], BF16, tag="xt", name=f"xt{i}") for i in range(NKD)]
    wqs = [p_wq.tile([128, DHC], BF16, tag="wq", name=f"wqs{i}") for i in range(NKD)]
    wks = [p_wk.tile([128, DHC], BF16, tag="wk", name=f"wks{i}") for i in range(NKD)]
    wvs = [p_wv.tile([128, DHC], BF16, tag="wv", name=f"wvs{i}") for i in range(NKD)]
    wos = [p_wo.tile([128, D], BF16, tag="wo", name=f"wos{i}") for i in range(4)]
    # first-needed chunks first: qk0_tt0 needs xt[:, 0:512] + w[:, 0:128]
    for kd in range(NKD):
        nc.sync.dma_start(xt[kd][:, 0:512], xT[kd * 128:(kd + 1) * 128, 0:512])
        nc.sync.dma_start(wqs[kd][:, 0:128], wqT[kd * 128:(kd + 1) * 128, 0:128])
        nc.sync.dma_start(wks[kd][:, 0:128], wkT[kd * 128:(kd + 1) * 128, 0:128])
    for kd in range(NKD):
        nc.sync.dma_start(xt[kd][:, 512:2048], xT[kd * 128:(kd + 1) * 128, 512:2048])
        nc.gpsimd.dma_start(wvs[kd][:], wvT[kd * 128:(kd + 1) * 128, :])
    for kd in range(NKD):
        nc.sync.dma_start(wqs[kd][:, 128:512], wqT[kd * 128:(kd + 1) * 128, 128:512])
        nc.sync.dma_start(wks[kd][:, 128:512], wkT[kd * 128:(kd + 1) * 128, 128:512])
    for j in range(4):
        nc.gpsimd.dma_start(wos[j][:], woT[j * 128:(j + 1) * 128, :])
    bqt, bkt = [], []
    for hp in range(NHP):
        tq = p_bias.tile([128, 1], F32, tag="bias")
        tk = p_bias.tile([128, 1], F32, tag="bias")
        nc.gpsimd.dma_start(tq[:], bq[hp * 128:(hp + 1) * 128, :])
        nc.gpsimd.dma_start(tk[:], bk[hp * 128:(hp + 1) * 128, :])
        bqt.append(tq)
        bkt.append(tk)
    ones_sb = p_bias.tile([128, 64], F32, tag="bias")
    nc.vector.memset(ones_sb[:], 1.0)

    q_t, k_t = [None] * NHP, [None] * NHP
    v_t = [None] * NTT
    outst_q = {}

    def _ensure_qk(hp):
        if q_t[hp] is None:
            q_t[hp] = p_q.tile([128, L], BF16, tag="q", name=f"qT{hp}")
            k_t[hp] = p_k.tile([128, L], BF16, tag="k", name=f"kT{hp}")

    def emit_q_unit(hp, tt, pool):
        _ensure_qk(hp)
        ps = pool.tile([128, 512], F32, tag=pool.name[1:], name=f"psq{hp}_{tt}")
        for kd in range(NKD):
            nc.tensor.matmul(
                ps[:], wqs[kd][:, hp * 128:(hp + 1) * 128],
                xt[kd][:, tt * 512:(tt + 1) * 512],
                start=(kd == 0), stop=(kd == NKD - 1),
            )
        nc.vector.tensor_scalar(
            q_t[hp][:, tt * 512:(tt + 1) * 512], ps[:], bqt[hp][:], None, ADD)

    def emit_k_unit(hp, tt, pool):
        _ensure_qk(hp)
        ps = pool.tile([128, 512], F32, tag=pool.name[1:], name=f"psk{hp}_{tt}")
        for kd in range(NKD):
            nc.tensor.matmul(
                ps[:], wks[kd][:, hp * 128:(hp + 1) * 128],
                xt[kd][:, tt * 512:(tt + 1) * 512],
                start=(kd == 0), stop=(kd == NKD - 1),
            )
        nc.vector.tensor_scalar(
            k_t[hp][:, tt * 512:(tt + 1) * 512], ps[:], bkt[hp][:], None, ADD)

    def emit_v(t, pool):
        ps = pool.tile([128, 512], F32, tag=pool.name[1:], name=f"psv{t}")
        for kd in range(NKD):
            nc.tensor.matmul(
                ps[:], xt[kd][:, t * 128:(t + 1) * 128], wvs[kd][:],
                start=(kd == 0), stop=(kd == NKD - 1),
            )
        vt = p_v.tile([128, HPC, HD + 1], BF16, tag="v", name=f"v{t}")
        nc.vector.memset(vt[:, :, HD:HD + 1], 1.0)
        nc.vector.tensor_copy(
            vt[:, :, 0:HD], ps[:].rearrange("p (h d) -> p h d", h=HPC))
        v_t[t] = vt

    def emit_outproj_chunk(qt, ost, tl, half):
        key = (qt, tl)
        if key not in outst_q:
            outst_q[key] = p_outst.tile(
                [128, 1024], F32, tag="outst", name=f"outst{qt}_{tl}")
        outst = outst_q[key]
        ps_o = p_bg.tile([128, 512], F32, tag="bg", name=f"pso{qt}_{tl}_{half}")
        for j in range(NHP):
            nc.tensor.matmul(
                ps_o[:],
                ost[j][:, tl * 128:(tl + 1) * 128],
                wos[j][:, half * 512:(half + 1) * 512],
                start=(j == 0), stop=(j == NHP - 1),
            )
        nc.vector.tensor_copy(outst[:, half * 512:(half + 1) * 512], ps_o[:])
        if half == 1:
            t = qt * 4 + tl
            nc.gpsimd.dma_start(out[t * 128:(t + 1) * 128, :], outst[:])

    bg = deque()

    def _pre_kt(kt):
        if bg:
            bg.popleft()()

    def emit_attn(qt, hp):
        po_a = p_acc.tile([128, 512], F32, tag="acc", name=f"poa{qt}_{hp}")
        po_b = p_acc.tile([128, 512], F32, tag="acc", name=f"pob{qt}_{hp}")
        for kt in range(NKT):
            _pre_kt(kt)
            # scores + exp are the ACT-feeding critical path: lift their
            # priority above backlogged attnv / background work so the
            # scheduler never starves ScalarE behind lower-value matmuls.
            with tc.high_priority(offset=300):
                ps = p_mm.tile([128, 1024], F32, tag="mm", name=f"pss{qt}_{hp}_{kt}")
                nc.tensor.matmul(
                    ps[:, 0:512],
                    k_t[hp][0:64, kt * 128:(kt + 1) * 128],
                    q_t[hp][0:64, qt * 512:(qt + 1) * 512],
                    start=True, stop=True,
                )
                nc.tensor.matmul(
                    ps[:, 512:1024],
                    k_t[hp][64:128, kt * 128:(kt + 1) * 128],
                    q_t[hp][64:128, qt * 512:(qt + 1) * 512],
                    start=True, stop=True,
                )
                e = p_e.tile([128, 1024], BF16, tag="e", name=f"e{qt}_{hp}_{kt}")
                nc.scalar.activation(e[:], ps[:], EXP)
            nc.tensor.matmul(
                po_a[0:65, :], v_t[kt][:, 2 * hp, :], e[:, 0:512],
                start=(kt == 0), stop=(kt == NKT - 1),
            )
            nc.tensor.matmul(
                po_b[0:65, :], v_t[kt][:, 2 * hp + 1, :], e[:, 512:1024],
                start=(kt == 0), stop=(kt == NKT - 1),
            )
        o = p_ost.tile([128, 512], BF16, tag=f"ost{hp}", name=f"ost{qt}_{hp}")
        for half, po in ((0, po_a), (1, po_b)):
            # approx_fast mishandles nonzero base partitions -> run over
            # [0:65] (base 0) and consume row 64 only; rows 0:64 are junk.
            rcp = p_rcp.tile([128, 512], F32, tag="rcp", name=f"rcp{qt}_{hp}_{half}")
            nc.vector.reciprocal_approx_fast(out=rcp[0:65, :], in_=po[0:65, :])
            # broadcast recip into the po bank's unused partitions 64:128
            # (row 64's sums are already consumed by the reciprocal)
            nc.tensor.matmul(
                po[64:128, :], ones_sb[64:65, :], rcp[64:65, :],
                start=True, stop=True, tile_position=(64, 64),
                skip_group_check=True,
            )
            rb = p_rb.tile([64, 512], F32, tag="rb", name=f"rb{qt}_{hp}_{half}")
            nc.vector.tensor_copy(rb[:], po[64:128, :])
            nc.vector.tensor_mul(o[64 * half:64 * half + 64, :], po[0:64, :], rb[:])
        return o

    # --- emission: jit prefix; per-block background units with deadlines ---
    ost_q = [[None] * NHP for _ in range(NQT)]

    def _pre_kt(kt):
        if bg:
            u = bg.popleft()
            if u is not None:
                u()

    def spaced(units, gap):
        seq = []
        for u in units:
            seq.append(u)
            seq.extend([None] * gap)
        return seq

    def Q(hp, tt):
        return lambda: emit_q_unit(hp, tt, p_bg)

    def K(hp, tt):
        return lambda: emit_k_unit(hp, tt, p_bg)

    def V(t):
        return lambda: emit_v(t, p_bg)

    def OP(qt, tl, half):
        return lambda: emit_outproj_chunk(qt, ost_q[qt], tl, half)

    emit_k_unit(0, 0, p_mm)
    emit_q_unit(0, 0, p_mm)
    for t in range(3):
        emit_v(t, p_mm)

    # bg unit lists per (row=hp, block=qt), EDF-ordered within each block
    bgq = {
        (0, 0): [V(3), V(4), K(0, 1), V(5), V(6), V(7), K(0, 2), V(8), V(9),
                 V(10), V(11), K(0, 3), V(12), V(13), V(14), V(15)],
        (0, 1): [Q(0, 1)] + spaced([K(1, 0), K(1, 1), K(1, 2), K(1, 3)], 2),
        (0, 2): [Q(0, 2)] + spaced([Q(1, 0)], 4),
        (0, 3): [Q(0, 3)],
        (1, 1): [Q(1, 1)] + spaced([K(2, 0), K(2, 1), K(2, 2), K(2, 3)], 2),
        (1, 2): [Q(1, 2)] + spaced([Q(2, 0)], 4),
        (1, 3): [Q(1, 3)],
        (2, 1): [Q(2, 1)] + spaced([K(3, 0), K(3, 1), K(3, 2), K(3, 3)], 2),
        (2, 2): [Q(2, 2)] + spaced([Q(3, 0)], 4),
        (2, 3): [Q(2, 3)],
        (3, 0): [Q(3, 1), None, Q(3, 2), None, Q(3, 3)],
        (3, 1): spaced([OP(0, tl, h) for tl in range(4) for h in range(2)], 1),
        (3, 2): spaced([OP(1, tl, h) for tl in range(4) for h in range(2)], 1),
        (3, 3): spaced([OP(2, tl, h) for tl in range(4) for h in range(2)], 1),
    }
    for hp in range(NHP):
        for qt in range(NQT):
            while bg:  # drain any unpopped units from the previous block
                u = bg.popleft()
                if u is not None:
                    u()
            bg.extend(bgq.get((hp, qt), []))
            ost_q[qt][hp] = emit_attn(qt, hp)
    while bg:
        u = bg.popleft()
        if u is not None:
            u()
    # final query tile: scores ring is idle now, use its big slots
    for tl in range(4):
        qt = NQT - 1
        outst = p_outst.tile([128, 1024], F32, tag="outst", name=f"outstF_{tl}")
        ps_o = p_mm.tile([128, 1024], F32, tag="mm", name=f"psoF_{tl}")
        for half in range(2):
            for j in range(NHP):
                nc.tensor.matmul(
                    ps_o[:, half * 512:(half + 1) * 512],
                    ost_q[qt][j][:, tl * 128:(tl + 1) * 128],
                    wos[j][:, half * 512:(half + 1) * 512],
                    start=(j == 0), stop=(j == NHP - 1),
                )
        nc.vector.tensor_copy(outst[:], ps_o[:])
        t = qt * 4 + tl
        nc.gpsimd.dma_start(out[t * 128:(t + 1) * 128, :], outst[:])


def _build_nc():
    nc = bacc.Bacc("TRN2", target_bir_lowering=False, debug=False, num_devices=NCORES)
    with tile.TileContext(nc) as tc, ExitStack() as ctx:
        _emit(nc, tc, ctx)
    nc.compile()
    return nc


def kernel(inputs, wq, bq, wk, bk, wv, bv, wo, bo):
    global _NC_CACHE, LAST_RESULTS
    if _NC_CACHE is None:
        _NC_CACHE = _build_nc()
    nc = _NC_CACHE

    inputs = np.asarray(inputs, dtype=np.float32)
    wq, bq, wk, bk = (np.asarray(a, dtype=np.float32) for a in (wq, bq, wk, bk))
    wv, bv, wo, bo = (np.asarray(a, dtype=np.float32) for a in (wv, bv, wo, bo))

    scale = np.float32(1.0 / np.sqrt(HD))
    wqT_full = np.ascontiguousarray(wq.T) * scale
    wkT_full = np.ascontiguousarray(wk.T)
    wvT_full = np.ascontiguousarray(wv.T)
    woT_full = np.ascontiguousarray(wo.T)

    in_maps = []
    for c in range(NCORES):
        b, g = c // 2, c % 2
        cols = slice(g * DHC, (g + 1) * DHC)
        in_maps.append({
            "xT": np.ascontiguousarray(inputs[b].T).astype(NPBF16),
            "wqT": np.ascontiguousarray(wqT_full[:, cols]).astype(NPBF16),
            "wkT": np.ascontiguousarray(wkT_full[:, cols]).astype(NPBF16),
            "wvT": np.ascontiguousarray(wvT_full[:, cols]).astype(NPBF16),
            "woT": np.ascontiguousarray(woT_full[cols, :]).astype(NPBF16),
            "bq": (bq[cols] * scale).astype(np.float32).reshape(DHC, 1),
            "bk": bk[cols].astype(np.float32).reshape(DHC, 1),
        })

    trace = bool(int(os.environ.get("BASS_KERNEL_TRACE", "0")))
    res = run_bass_kernel_spmd(nc, in_maps, core_ids=list(range(NCORES)), trace=trace)
    LAST_RESULTS = res

    fused_bias = (bo + bv @ wo.T).astype(np.float32)
    outv = np.empty((B, L, D), dtype=np.float32)
    for b in range(B):
        outv[b] = res.results[2 * b]["out"] + res.results[2 * b + 1]["out"] + fused_bias
    return outv


# revision 20
# speedup vs baseline: 1.1117x; 1.0401x over previous
"""Multi-head self-attention (B=4, L=2048, D=1024, H=16, Hd=64) on 8 TRN2 NeuronCores.

Sharding: data-parallel over batch (4) x tensor-parallel over head-groups (2).
Core c handles batch c//2 with heads [8*(c%2), 8*(c%2)+8). Each core computes a
partial out-projection over its 512 head-dims; the host sums the two partials
per batch and adds the fused bias (bo + bv @ wo.T, exact because softmax rows
sum to 1, so the v-bias passes through attention unchanged).

Per-core kernel (bf16 compute, f32 accumulation):
  - q/k feature-major [512, 2048] = w.T-slice @ x.T (bias per-partition, q
    pre-scaled by 1/sqrt(Hd) via host-scaled weights)
  - v token-major [2048, 8*65] with a ones-column per head: the attn@v matmul
    (lhsT = v_aug) then yields softmax denominators in PSUM row 64 for free
  - scores S^T[key, q] per head via K=64 matmuls, two heads packed into the
    128x128 PE array via base-partition 0/64 row tiling
  - exp on ScalarE (no max-subtraction: scores are ~N(0,1), fp32-safe)
  - normalize: DVE reciprocal of row 64 -> PE ones-broadcast -> DVE scale
  - out-projection token-major, host adds partials
"""
import os
import sys
import time
from contextlib import ExitStack

sys.path.insert(0, "/opt/trn_rl_repo")

import numpy as np
import ml_dtypes

import concourse.bass as bass
import concourse.tile as tile
from concourse import bacc, mybir
from concourse.bass_utils import run_bass_kernel_spmd

BF16 = mybir.dt.bfloat16
F32 = mybir.dt.float32
NPBF16 = ml_dtypes.bfloat16

B, L, D = 4, 2048, 1024
H, HD = 16, 64
HPC = 8            # heads per core
DHC = HPC * HD     # 512 local head-dims per core
NCORES = 8

NKD = D // 128     # 8 contraction tiles over model dim
NTT = L // 128     # 16 token tiles of 128
NQT = L // 512     # 4 query tiles of 512
NKT = L // 128     # 16 key tiles of 128
NHP = HPC // 2     # 4 head pairs

_NC_CACHE = None
LAST_RESULTS = None  # test harness introspection


def _emit(nc, tc, ctx):
    EXP = mybir.ActivationFunctionType.Exp
    ADD = mybir.AluOpType.add
    from collections import deque

    xT = nc.declare_dram_parameter("xT", [D, L], BF16, isOutput=False)
    wqT = nc.declare_dram_parameter("wqT", [D, DHC], BF16, isOutput=False)
    wkT = nc.declare_dram_parameter("wkT", [D, DHC], BF16, isOutput=False)
    wvT = nc.declare_dram_parameter("wvT", [D, DHC], BF16, isOutput=False)
    woT = nc.declare_dram_parameter("woT", [DHC, D], BF16, isOutput=False)
    bq = nc.declare_dram_parameter("bq", [DHC, 1], F32, isOutput=False)
    bk = nc.declare_dram_parameter("bk", [DHC, 1], F32, isOutput=False)
    out = nc.declare_dram_parameter("out", [L, D], F32, isOutput=True)

    p_xt = ctx.enter_context(tc.tile_pool(name="xt", bufs=NKD))
    p_wq = ctx.enter_context(tc.tile_pool(name="wq", bufs=NKD))
    p_wk = ctx.enter_context(tc.tile_pool(name="wk", bufs=NKD))
    p_wv = ctx.enter_context(tc.tile_pool(name="wv", bufs=NKD))
    p_wo = ctx.enter_context(tc.tile_pool(name="wo", bufs=4))
    p_bias = ctx.enter_context(tc.tile_pool(name="bias", bufs=2 * NHP + 1))
    p_q = ctx.enter_context(tc.tile_pool(name="q", bufs=NHP))
    p_k = ctx.enter_context(tc.tile_pool(name="k", bufs=NHP))
    p_v = ctx.enter_context(tc.tile_pool(name="v", bufs=NTT))
    p_e = ctx.enter_context(tc.tile_pool(name="e", bufs=8))
    p_ost = ctx.enter_context(tc.tile_pool(name="ost", bufs=NQT + 1))
    p_rcp = ctx.enter_context(tc.tile_pool(name="rcp", bufs=2))
    p_rb = ctx.enter_context(tc.tile_pool(name="rb", bufs=2))
    p_outst = ctx.enter_context(tc.tile_pool(name="outst", bufs=3))
    # PSUM: scores 2x[128,1024] (4 banks) + attnv accum 2x[128,512] (2 banks)
    # + one background ring [128,1024] (2 banks) = 8 banks exactly.
    p_mm = ctx.enter_context(tc.tile_pool(name="pmm", bufs=2, space="PSUM"))
    p_acc = ctx.enter_context(tc.tile_pool(name="pacc", bufs=2, space="PSUM"))
    p_bg = ctx.enter_context(tc.tile_pool(name="pbg", bufs=2, space="PSUM"))

    # --- weight / input DMAs ---
    xt = [p_xt.tile([128, L], BF16, tag="xt", name=f"xt{i}") for i in range(NKD)]
    wqs = [p_wq.tile([128, DHC], BF16, tag="wq", name=f"wqs{i}") for i in range(NKD)]
    wks = [p_wk.tile([128, DHC], BF16, tag="wk", name=f"wks{i}") for i in range(NKD)]
    wvs = [p_wv.tile([128, DHC], BF16, tag="wv", name=f"wvs{i}") for i in range(NKD)]
    wos = [p_wo.tile([128, D], BF16, tag="wo", name=f"wos{i}") for i in range(4)]
    # first-needed chunks first: qk0_tt0 needs xt[:, 0:512] + w[:, 0:128]
    for kd in range(NKD):
        nc.sync.dma_start(xt[kd][:, 0:512], xT[kd * 128:(kd + 1) * 128, 0:512])
        nc.sync.dma_start(wqs[kd][:, 0:128], wqT[kd * 128:(kd + 1) * 128, 0:128])
        nc.sync.dma_start(wks[kd][:, 0:128], wkT[kd * 128:(kd + 1) * 128, 0:128])
    for kd in range(NKD):
        nc.sync.dma_start(xt[kd][:, 512:2048], xT[kd * 128:(kd + 1) * 128, 512:2048])
        nc.gpsimd.dma_start(wvs[kd][:], wvT[kd * 128:(kd + 1) * 128, :])
    for kd in range(NKD):
        nc.sync.dma_start(wqs[kd][:, 128:512], wqT[kd * 128:(kd + 1) * 128, 128:512])
        nc.sync.dma_start(wks[kd][:, 128:512], wkT[kd * 128:(kd + 1) * 128, 128:512])
    for j in range(4):
        nc.gpsimd.dma_start(wos[j][:], woT[j * 128:(j + 1) * 128, :])
    bqt, bkt = [], []
    for hp in range(NHP):
        tq = p_bias.tile([128, 1], F32, tag="bias")
        tk = p_bias.tile([128, 1], F32, tag="bias")
        nc.gpsimd.dma_start(tq[:], bq[hp * 128:(hp + 1) * 128, :])
        nc.gpsimd.dma_start(tk[:], bk[hp * 128:(hp + 1) * 128, :])
        bqt.append(tq)
        bkt.append(tk)
    ones_sb = p_bias.tile([128, 64], F32, tag="bias")
    nc.vector.memset(ones_sb[:], 1.0)

    q_t, k_t = [None] * NHP, [None] * NHP
    v_t = [None] * NTT
    outst_q = {}

    def _ensure_qk(hp):
        if q_t[hp] is None:
            q_t[hp] = p_q.tile([128, L], BF16, tag="q", name=f"qT{hp}")
            k_t[hp] = p_k.tile([128, L], BF16, tag="k", name=f"kT{hp}")

    def emit_q_unit(hp, tt, pool):
        _ensure_qk(hp)
        ps = pool.tile([128, 512], F32, tag=pool.name[1:], name=f"psq{hp}_{tt}")
        for kd in range(NKD):
            nc.tensor.matmul(
                ps[:], wqs[kd][:, hp * 128:(hp + 1) * 128],
                xt[kd][:, tt * 512:(tt + 1) * 512],
                start=(kd == 0), stop=(kd == NKD - 1),
            )
        with tc.high_priority(offset=300):
            nc.vector.tensor_scalar(
                q_t[hp][:, tt * 512:(tt + 1) * 512], ps[:], bqt[hp][:], None, ADD)

    def emit_k_unit(hp, tt, pool):
        _ensure_qk(hp)
        ps = pool.tile([128, 512], F32, tag=pool.name[1:], name=f"psk{hp}_{tt}")
        for kd in range(NKD):
            nc.tensor.matmul(
                ps[:], wks[kd][:, hp * 128:(hp + 1) * 128],
                xt[kd][:, tt * 512:(tt + 1) * 512],
                start=(kd == 0), stop=(kd == NKD - 1),
            )
        with tc.high_priority(offset=300):
            nc.vector.tensor_scalar(
                k_t[hp][:, tt * 512:(tt + 1) * 512], ps[:], bkt[hp][:], None, ADD)

    def emit_v(t, pool):
        ps = pool.tile([128, 512], F32, tag=pool.name[1:], name=f"psv{t}")
        for kd in range(NKD):
            nc.tensor.matmul(
                ps[:], xt[kd][:, t * 128:(t + 1) * 128], wvs[kd][:],
                start=(kd == 0), stop=(kd == NKD - 1),
            )
        vt = p_v.tile([128, HPC, HD + 1], BF16, tag="v", name=f"v{t}")
        with tc.high_priority(offset=300):
            nc.vector.memset(vt[:, :, HD:HD + 1], 1.0)
            nc.vector.tensor_copy(
                vt[:, :, 0:HD], ps[:].rearrange("p (h d) -> p h d", h=HPC))
        v_t[t] = vt

    def emit_outproj_chunk(qt, ost, tl, half):
        key = (qt, tl)
        if key not in outst_q:
            outst_q[key] = p_outst.tile(
                [128, 1024], F32, tag="outst", name=f"outst{qt}_{tl}")
        outst = outst_q[key]
        ps_o = p_bg.tile([128, 512], F32, tag="bg", name=f"pso{qt}_{tl}_{half}")
        for j in range(NHP):
            nc.tensor.matmul(
                ps_o[:],
                ost[j][:, tl * 128:(tl + 1) * 128],
                wos[j][:, half * 512:(half + 1) * 512],
                start=(j == 0), stop=(j == NHP - 1),
            )
        nc.vector.tensor_copy(outst[:, half * 512:(half + 1) * 512], ps_o[:])
        if half == 1:
            t = qt * 4 + tl
            nc.gpsimd.dma_start(out[t * 128:(t + 1) * 128, :], outst[:])

    bg = deque()

    def _pre_kt(kt):
        if bg:
            bg.popleft()()

    def emit_attn(qt, hp):
        po_a = p_acc.tile([128, 512], F32, tag="acc", name=f"poa{qt}_{hp}")
        po_b = p_acc.tile([128, 512], F32, tag="acc", name=f"pob{qt}_{hp}")
        for kt in range(NKT):
            _pre_kt(kt)
            # scores + exp are the ACT-feeding critical path: lift their
            # priority above backlogged attnv / background work so the
            # scheduler never starves ScalarE behind lower-value matmuls.
            with tc.high_priority(offset=300):
                ps = p_mm.tile([128, 1024], F32, tag="mm", name=f"pss{qt}_{hp}_{kt}")
                nc.tensor.matmul(
                    ps[:, 0:512],
                    k_t[hp][0:64, kt * 128:(kt + 1) * 128],
                    q_t[hp][0:64, qt * 512:(qt + 1) * 512],
                    start=True, stop=True,
                )
                nc.tensor.matmul(
                    ps[:, 512:1024],
                    k_t[hp][64:128, kt * 128:(kt + 1) * 128],
                    q_t[hp][64:128, qt * 512:(qt + 1) * 512],
                    start=True, stop=True,
                )
                e = p_e.tile([128, 1024], BF16, tag="e", name=f"e{qt}_{hp}_{kt}")
                nc.scalar.activation(e[:], ps[:], EXP)
            nc.tensor.matmul(
                po_a[0:65, :], v_t[kt][:, 2 * hp, :], e[:, 0:512],
                start=(kt == 0), stop=(kt == NKT - 1),
            )
            nc.tensor.matmul(
                po_b[0:65, :], v_t[kt][:, 2 * hp + 1, :], e[:, 512:1024],
                start=(kt == 0), stop=(kt == NKT - 1),
            )
        o = p_ost.tile([128, 512], BF16, tag=f"ost{hp}", name=f"ost{qt}_{hp}")
        for half, po in ((0, po_a), (1, po_b)):
            # approx_fast mishandles nonzero base partitions -> run over
            # [0:65] (base 0) and consume row 64 only; rows 0:64 are junk.
            rcp = p_rcp.tile([128, 512], F32, tag="rcp", name=f"rcp{qt}_{hp}_{half}")
            nc.vector.reciprocal_approx_fast(out=rcp[0:65, :], in_=po[0:65, :])
            # broadcast recip into the po bank's unused partitions 64:128
            # (row 64's sums are already consumed by the reciprocal)
            nc.tensor.matmul(
                po[64:128, :], ones_sb[64:65, :], rcp[64:65, :],
                start=True, stop=True, tile_position=(64, 64),
                skip_group_check=True,
            )
            rb = p_rb.tile([64, 512], F32, tag="rb", name=f"rb{qt}_{hp}_{half}")
            nc.vector.tensor_copy(rb[:], po[64:128, :])
            nc.vector.tensor_mul(o[64 * half:64 * half + 64, :], po[0:64, :], rb[:])
        return o

    # --- emission: jit prefix; per-block background units with deadlines ---
    ost_q = [[None] * NHP for _ in range(NQT)]

    def _pre_kt(kt):
        if bg:
            u = bg.popleft()
            if u is not None:
                u()

    def spaced(units, gap):
        seq = []
        for u in units:
            seq.append(u)
            seq.extend([None] * gap)
        return seq

    def Q(hp, tt):
        return lambda: emit_q_unit(hp, tt, p_bg)

    def K(hp, tt):
        return lambda: emit_k_unit(hp, tt, p_bg)

    def V(t):
        return lambda: emit_v(t, p_bg)

    def OP(qt, tl, half):
        return lambda: emit_outproj_chunk(qt, ost_q[qt], tl, half)

    emit_k_unit(0, 0, p_mm)
    emit_q_unit(0, 0, p_mm)
    for t in range(3):
        emit_v(t, p_mm)

    # bg unit lists per (row=hp, block=qt), EDF-ordered within each block
    bgq = {
        (0, 0): [V(3), V(4), K(0, 1), V(5), V(6), V(7), K(0, 2), V(8), V(9),
                 V(10), V(11), K(0, 3), V(12), V(13), V(14), V(15)],
        (0, 1): [Q(0, 1), K(1, 0), None, K(1, 1), None, K(1, 2), None,
                 K(1, 3), None, Q(0, 2), None, Q(0, 3)],
        (0, 2): [Q(1, 0), None, None, None, Q(1, 1)],
        (0, 3): [Q(1, 2), None, None, None, Q(1, 3)],
        (1, 0): spaced([K(2, 0), K(2, 1)], 3),
        (1, 1): spaced([K(2, 2), K(2, 3), Q(2, 0)], 3),
        (1, 2): spaced([Q(2, 1), Q(2, 2)], 4),
        (1, 3): spaced([Q(2, 3)], 4),
        (2, 0): spaced([K(3, 0), K(3, 1)], 3),
        (2, 1): spaced([K(3, 2), K(3, 3), Q(3, 0)], 3),
        (2, 2): spaced([Q(3, 1), Q(3, 2)], 4),
        (2, 3): spaced([Q(3, 3)], 4),
        (3, 1): spaced([OP(0, tl, h) for tl in range(4) for h in range(2)], 1),
        (3, 2): spaced([OP(1, tl, h) for tl in range(4) for h in range(2)], 1),
        (3, 3): spaced([OP(2, tl, h) for tl in range(4) for h in range(2)], 1),
    }
    for hp in range(NHP):
        for qt in range(NQT):
            while bg:  # drain any unpopped units from the previous block
                u = bg.popleft()
                if u is not None:
                    u()
            bg.extend(bgq.get((hp, qt), []))
            ost_q[qt][hp] = emit_attn(qt, hp)
    while bg:
        u = bg.popleft()
        if u is not None:
            u()
    # final query tile: scores ring is idle now, use its big slots
    for tl in range(4):
        qt = NQT - 1
        outst = p_outst.tile([128, 1024], F32, tag="outst", name=f"outstF_{tl}")
        ps_o = p_mm.tile([128, 1024], F32, tag="mm", name=f"psoF_{tl}")
        for half in range(2):
            for j in range(NHP):
                nc.tensor.matmul(
                    ps_o[:, half * 512:(half + 1) * 512],
                    ost_q[qt][j][:, tl * 128:(tl + 1) * 128],
                    wos[j][:, half * 512:(half + 1) * 512],
                    start=(j == 0), stop=(j == NHP - 1),
                )
        nc.vector.tensor_copy(outst[:], ps_o[:])
        t = qt * 4 + tl
        nc.gpsimd.dma_start(out[t * 128:(t + 1) * 128, :], outst[:])


def _build_nc():
    nc = bacc.Bacc("TRN2", target_bir_lowering=False, debug=False, num_devices=NCORES)
    with tile.TileContext(nc) as tc, ExitStack() as ctx:
        _emit(nc, tc, ctx)
    nc.compile()
    return nc


def kernel(inputs, wq, bq, wk, bk, wv, bv, wo, bo):
    global _NC_CACHE, LAST_RESULTS
    if _NC_CACHE is None:
        _NC_CACHE = _build_nc()
    nc = _NC_CACHE

    inputs = np.asarray(inputs, dtype=np.float32)
    wq, bq, wk, bk = (np.asarray(a, dtype=np.float32) for a in (wq, bq, wk, bk))
    wv, bv, wo, bo = (np.asarray(a, dtype=np.float32) for a in (wv, bv, wo, bo))

    scale = np.float32(1.0 / np.sqrt(HD))
    wqT_full = np.ascontiguousarray(wq.T) * scale
    wkT_full = np.ascontiguousarray(wk.T)
    wvT_full = np.ascontiguousarray(wv.T)
    woT_full = np.ascontiguousarray(wo.T)

    in_maps = []
    for c in range(NCORES):
        b, g = c // 2, c % 2
        cols = slice(g * DHC, (g + 1) * DHC)
        in_maps.append({
            "xT": np.ascontiguousarray(inputs[b].T).astype(NPBF16),
            "wqT": np.ascontiguousarray(wqT_full[:, cols]).astype(NPBF16),
            "wkT": np.ascontiguousarray(wkT_full[:, cols]).astype(NPBF16),
            "wvT": np.ascontiguousarray(wvT_full[:, cols]).astype(NPBF16),
            "woT": np.ascontiguousarray(woT_full[cols, :]).astype(NPBF16),
            "bq": (bq[cols] * scale).astype(np.float32).reshape(DHC, 1),
            "bk": bk[cols].astype(np.float32).reshape(DHC, 1),
        })

    trace = bool(int(os.environ.get("BASS_KERNEL_TRACE", "0")))
    res = run_bass_kernel_spmd(nc, in_maps, core_ids=list(range(NCORES)), trace=trace)
    LAST_RESULTS = res

    fused_bias = (bo + bv @ wo.T).astype(np.float32)
    outv = np.empty((B, L, D), dtype=np.float32)
    for b in range(B):
        outv[b] = res.results[2 * b]["out"] + res.results[2 * b + 1]["out"] + fused_bias
    return outv
